# revision 39
# baseline (speedup 1.0000x reference)
"""Classical self-attention on 8 trn2 NeuronCores — v3.

N=16384 tokens, d=64, fp32. Sequence-parallel over Q: core c handles rows
[c*2048, (c+1)*2048). Per-core differentiation via the xq input slice.

Algebra (host-folded):
  s_jq = k_j . q_q = x_j^T (G x_q + w) + const_q, G/w precomputed on host;
  the per-q const is absorbed by the softmax shift, so it is never computed.
  sum_j p_j v_j = [Wv|bv] @ (sum_j p_j [x_j;1])  -> V never materialized;
  the ones column of xh doubles as the denominator row.

Structure per core:
  setup: DMA x (8 chunks) + xq; PE-transpose x -> xT[65,16384] (ones row);
    fp8 shadows x8[32,2,N] (DoubleRow d-split: d = slot*32 + partition);
    g-proj gB[65,2048] = [G^T;w^T] @ xqT, fp8 shadow g8; pass A for group 0.
  pass A (row max): fp8 DR matmuls psA[128,1024] per (tile, 1024-key chunk);
    DVE tensor_tensor_reduce (max,max) -> mms[:,t,c]; mfin: reduce -> mt,
    PE transpose -> gB row 64 = -m̂.
  8 phases (groups of 256 q = 2 tiles): per unit (4 kv blocks):
    PV(u-2) x4, B x4 (fp32r, psB[:,u%2,:]), exp(u-1) on ACT, TTR(u-1),
    A-mm(u) for group h+1 last (spacing hides the single-buffered psA WAR).
  tail: psPV[65,256] -> OT; after phase 7: OTw = [Wv|bv]^T @ OT (+denom row),
    PE transpose, DVE reciprocal+scale, DMA out.
"""

import sys

sys.path.insert(0, "/opt/trn_rl_repo")

from contextlib import ExitStack

import numpy as np

import concourse.bass as bass
import concourse.mybir as mybir
import concourse.tile as tile
from concourse import bacc
from concourse.bass import ds, ts
from concourse.bass_utils import run_bass_kernel_spmd

N_CORES = 8
N = 16384
D = 64
QR = N // N_CORES          # 2048 q rows per core
N_QTILE = QR // 128        # 16 q tiles per core
N_PHASE = 8                # groups of 2 q-tiles (256 q cols)
N_KV_BLK = N // 128        # 128 kv blocks
N_UNIT = 32                # units per phase (4 blocks each)
N_ACH = 16                 # 1024-key A-chunks per q tile
F32 = mybir.dt.float32
F8 = mybir.dt.float8e4
R32 = mybir.dt.float32r
DR = mybir.MatmulPerfMode.DoubleRow
AX = mybir.AxisListType.X
MAX = mybir.AluOpType.max

_CACHED = {}


def build_kernel():
    nc = bacc.Bacc("TRN2", target_bir_lowering=False, debug=False,
                   num_devices=N_CORES)

    x_d = nc.dram_tensor("x", [N, D], F32, kind="ExternalInput")
    xq_d = nc.dram_tensor("xq", [QR, D], F32, kind="ExternalInput")
    gm_d = nc.dram_tensor("gmat", [D + 1, D], F32, kind="ExternalInput")
    wv_d = nc.dram_tensor("wvb", [D + 1, D], F32, kind="ExternalInput")
    id_d = nc.dram_tensor("ident", [128, 128], F32, kind="ExternalInput")
    nid_d = nc.dram_tensor("nident", [128, 128], F32, kind="ExternalInput")
    oc_d = nc.dram_tensor("onescol", [128, 128], F32, kind="ExternalInput")
    y_d = nc.dram_tensor("y", [QR, D], F32, kind="ExternalOutput")

    def rb(ap):
        return ap.bitcast(R32)

    with tile.TileContext(nc) as tc, ExitStack() as ctx:
        sb = ctx.enter_context(tc.tile_pool(name="sb", bufs=1))
        expp = ctx.enter_context(tc.tile_pool(name="expp", bufs=4))
        smp = ctx.enter_context(tc.tile_pool(name="smp", bufs=4))
        scrp = ctx.enter_context(tc.tile_pool(name="scr", bufs=2))
        psB_p = ctx.enter_context(tc.tile_pool(name="psB", bufs=1, space="PSUM"))
        psA_p = ctx.enter_context(tc.tile_pool(name="psA", bufs=1, space="PSUM"))
        psPV_p = ctx.enter_context(tc.tile_pool(name="psPV", bufs=1, space="PSUM"))
        psM_p = ctx.enter_context(tc.tile_pool(name="psM", bufs=1, space="PSUM"))

        # ---- persistent SBUF ----
        xT = sb.tile([D + 1, N], F32)          # x^T, row 64 = ones
        x8 = sb.tile([32, 2, N], F8)           # DR d-split shadow of xT
        xV = sb.tile([128, N_KV_BLK, D + 1], F32)  # natural x, col 64 = ones
        xqn = sb.tile([128, N_QTILE, D + 1], F32)  # natural xq + ones col
        xqT = sb.tile([D + 1, QR], F32)        # xq^T, row 64 = ones
        gB = sb.tile([D + 1, QR], F32)         # g, row 64 = -m̂
        g8 = sb.tile([32, 2, QR], F8)
        gmat = sb.tile([D + 1, D], F32)
        wvb = sb.tile([D + 1, D], F32)
        ident = sb.tile([128, 128], F32)
        nident = sb.tile([128, 128], F32)
        onesb = sb.tile([128, 128], F32)
        mms = sb.tile([128, N_QTILE, 24], F32)
        # OT rows 0:64: out1 = sum p x, overwritten in-place by the
        # [Wv|bv]-stage at each phase tail; row 64 = denominator (sum p).
        OT = sb.tile([D + 1, QR], F32)
        y_sb = sb.tile([128, N_QTILE, D], F32)

        # ---- PSUM ----
        # The tile framework tracks PSUM hazards at tile granularity (and
        # marks PSUM-reading ACT/DVE ops as writers), so double-buffering
        # must use SEPARATE tiles, not halves of one tile.
        psBs = [psB_p.tile([128, 1024], F32, tag="psba", name="psba"),
                psB_p.tile([128, 1024], F32, tag="psbb", name="psbb")]
        psAb = psA_p.tile([128, 1024], F32, tag="psab", name="psab")
        psAsm = psA_p.tile([128, 512], F32, tag="psas", name="psas")
        # psPV bank also hosts pneg (partition 127) - disjoint regions of
        # one tile; tile-granular hazards only serialize phase tails.
        psPV = psPV_p.tile([128, 512], F32)

        # ---- DMAs (DMA_ENGINES serializes; order by first use) ----
        nc.gpsimd.dma_start(ident[:], id_d[:])
        nc.gpsimd.dma_start(gmat[:].bitcast(R32), gm_d[:].bitcast(R32))
        nc.gpsimd.dma_start(wvb[:], wv_d[:])
        nc.gpsimd.dma_start(nident[:], nid_d[:])
        nc.gpsimd.dma_start(onesb[:], oc_d[:])
        nc.sync.dma_start(
            xqn[:, :, 0:D], xq_d[:].rearrange("(j p) d -> p j d", p=128))
        nc.gpsimd.memset(xqn[:, :, D : D + 1], 1.0)
        for c in range(8):
            nc.sync.dma_start(
                xV[:, ds(c * 16, 16), 0:D].bitcast(R32),
                x_d[ds(c * 2048, 2048), :].bitcast(R32).rearrange(
                    "(j p) d -> p j d", p=128))
        # r32 memset fails the ISA check; copy a DMA-loaded ones tile
        nc.vector.tensor_copy(rb(xV[:, :, D : D + 1]), onesb[:, :].rearrange("p (j o) -> p j o", o=1))

        # ---- xq transposes (borrow psB halves) + g-proj + g8 ----
        for o in range(2):
            tgt = psBs[o % 2]
            for j in range(8):
                nc.tensor.transpose(tgt[0 : D + 1, ts(j, 128)],
                                    xqn[:, o * 8 + j, :], ident[:])
            nc.vector.tensor_copy(rb(xqT[:, ts(o, 1024)]), tgt[0 : D + 1, :])
        for s in range(8):
            pm = psAb[0:D, ds((s % 2) * 256, 256)]
            nc.tensor.matmul(pm, rb(gmat[:]), rb(xqT[:, ts(s, 256)]),
                             start=True, stop=True)
            nc.vector.tensor_copy(rb(gB[0:D, ts(s, 256)]), pm)
        nc.scalar.copy(g8[:, 0, :], gB[0:32, :])
        nc.scalar.copy(g8[:, 1, :], gB[32:64, :])

        # ---- pass-A plumbing ----
        # DVE must scan all N keys per q tile (hw: one PSUM operand per DVE
        # op, so no pairing); big 1024-key chunks amortize access overhead,
        # a second 512-key tile keeps the psA WAR chains off the PE queue.
        # Even q tiles take big chunks from keys [0,8K), odd from [8K,16K).
        def a_off(t, big):
            return (0 if t % 2 == 0 else 8192) if big else                    (8192 if t % 2 == 0 else 0)

        def emit_A_big(t, c):
            nc.tensor.matmul(psAb[:], g8[:, :, ts(t, 128)],
                             x8[:, :, ds(a_off(t, True) + c * 1024, 1024)],
                             start=True, stop=True, perf_mode=DR)

        def emit_red_big(t, c):
            nc.vector.reduce_max(mms[:, t, c : c + 1], psAb[:], axis=AX)

        def emit_A_sm(t, j):
            nc.tensor.matmul(psAsm[:], g8[:, :, ts(t, 128)],
                             x8[:, :, ds(a_off(t, False) + j * 512, 512)],
                             start=True, stop=True, perf_mode=DR)

        def emit_red_sm(t, j):
            nc.vector.reduce_max(mms[:, t, 8 + j : 9 + j], psAsm[:], axis=AX)

        def emit_mfin(t):
            mt = smp.tile([128, 1], F32, tag="mt")
            nc.vector.reduce_max(mt[:], mms[:, t, :], axis=AX)
            # pneg = mt^T @ (-I) = -m̂ row; plain DVE copy into gB row 64
            # keeps the hot ACT queue out of the phase-boundary chain.
            pneg = psPV[64:65, 256:384]
            nc.tensor.matmul(pneg, mt[:], nident[:], start=True, stop=True)
            nc.vector.tensor_copy(rb(gB[D : D + 1, ts(t, 128)]), pneg)

        # ---- setup: kv chunks -> xT, x8, pass A for group 0 ----
        for c in range(8):
            for oo in range(2):
                o = 2 * c + oo
                tgt = psBs[o % 2]
                for j in range(8):
                    nc.tensor.transpose(tgt[0 : D + 1, ts(j, 128)],
                                        xV[:, o * 8 + j, :], ident[:])
                nc.scalar.copy(rb(xT[:, ts(o, 1024)]), tgt[0 : D + 1, :])
                nc.scalar.copy(x8[:, 0, ts(o, 1024)], tgt[0:32, :])
                # Pool cannot read PSUM: slot-1 shadow reads xT after its copy
                nc.gpsimd.tensor_copy(x8[:, 1, ts(o, 1024)],
                                      xT[32:64, ts(o, 1024)])
                # group-0 A work: oct o = keys [1024*o, 1024*o+1024).
                # tile 0 (even): big for o<8, sm pairs for o>=8; tile 1
                # (odd): the reverse. One big + two sm per oct.
                if o < 8:
                    emit_A_big(0, o)
                    emit_red_big(0, o)
                    emit_A_sm(1, 2 * o)
                    emit_red_sm(1, 2 * o)
                    emit_A_sm(1, 2 * o + 1)
                    emit_red_sm(1, 2 * o + 1)
                else:
                    emit_A_big(1, o - 8)
                    emit_red_big(1, o - 8)
                    emit_A_sm(0, 2 * (o - 8))
                    emit_red_sm(0, 2 * (o - 8))
                    emit_A_sm(0, 2 * (o - 8) + 1)
                    emit_red_sm(0, 2 * (o - 8) + 1)
        emit_mfin(0)
        emit_mfin(1)

        # ---- phases ----
        for h in range(N_PHASE):
            qsl = ds(h * 256, 256)
            atiles = (2 * (h + 1), 2 * (h + 1) + 1) if h < N_PHASE - 1 else None
            pv_q = []   # batches of (blk, ex_ap); popped two units later
            # A-work queue: per tile 8 big + 16 sm ops; interleave
            # big/sm so the single-buffered tiles get WAR slack. Front-load
            # slightly so mfin chains clear before the next phase starts.
            a_q = []
            if atiles is not None:
                for t in atiles:
                    for c in range(8):
                        a_q.append(("big", t, c))
                        a_q.append(("sm", t, 2 * c))
                        a_q.append(("sm", t, 2 * c + 1))
            a_done = {}

            def emit_exp(u):
                ex = expp.tile([128, 1024], F32, tag="ex")
                nc.scalar.activation(rb(ex[:]), psBs[u % 2][:],
                                     mybir.ActivationFunctionType.Exp)
                pv_q.append([(4 * u + j, ex[:, ds(j * 256, 256)])
                             for j in range(4)])

            def emit_pv():
                for blk, ex_ap in pv_q.pop(0):
                    nc.tensor.matmul(psPV[0 : D + 1, 0:256], rb(xV[:, blk, :]),
                                     rb(ex_ap), start=(blk == 0),
                                     stop=(blk == N_KV_BLK - 1),
                                     skip_group_check=True)

            for u in range(N_UNIT):
                # Bs first: they feed this unit's exp with no other deps.
                for j in range(4):
                    blk = 4 * u + j
                    nc.tensor.matmul(psBs[u % 2][:, ds(j * 256, 256)],
                                     rb(xT[:, ts(blk, 128)]), rb(gB[:, qsl]),
                                     start=True, stop=True)
                # PVs of unit u-2: their exp finished during unit u-1.
                if len(pv_q) >= 2:
                    emit_pv()
                n_a = 2 if u < 8 else (2 if u % 2 == 0 else 1)
                for _ in range(min(n_a, len(a_q))):
                    kind, t, k = a_q.pop(0)
                    if kind == "big":
                        emit_A_big(t, k)
                        emit_red_big(t, k)
                    else:
                        emit_A_sm(t, k)
                        emit_red_sm(t, k)
                    a_done[t] = a_done.get(t, 0) + 1
                    if a_done[t] == 24:
                        emit_mfin(t)
                emit_exp(u)
            # tail: mfin chains are already emitted; flush PVs, save out1
            while pv_q:
                emit_pv()
            nc.vector.tensor_copy(rb(OT[:, qsl]), psPV[0 : D + 1, 0:256])
            # [Wv|bv]-stage for this group, written back into OT rows 0:64;
            # then the denominator row becomes its reciprocal in place, so
            # the final transpose carries 1/denom in column 64. Exact fp32:
            # 8 matmuls of 256 cols, precision is worth the 4x row cost.
            pm = psAb[0:D, 256:512]
            nc.tensor.matmul(pm, wvb[:], OT[:, qsl],
                             start=True, stop=True)
            nc.vector.tensor_copy(rb(OT[0:D, qsl]), pm)
            # reciprocal can't emit fp32r; bounce through scratch + rb copy
            rrow = scrp.tile([128, 256], F32, tag="scr")
            nc.vector.reciprocal(rrow[0:1, :], OT[D : D + 1, qsl])
            nc.vector.tensor_copy(rb(OT[D : D + 1, qsl]), rrow[0:1, :])

        # ---- final: transpose OT (col 64 = 1/denom), normalize, store ----
        for t in range(N_QTILE):
            pO = psBs[t % 2][:, 0 : D + 1]
            nc.tensor.transpose(pO, OT[:, ts(t, 128)],
                                ident[0 : D + 1, 0 : D + 1])
            nc.vector.tensor_scalar_mul(y_sb[:, t, :], pO[:, 0:D],
                                        pO[:, D : D + 1])
            if t % 4 == 3:
                nc.sync.dma_start(
                    y_d[ds((t - 3) * 128, 512), :].rearrange(
                        "(t p) d -> p t d", p=128),
                    y_sb[:, ds(t - 3, 4), :])

    nc.compile()
    return nc


def _prep_inputs(x, params, Wq, bq, Wk, bk, Wv, bv):
    f8s = np.float64
    x = np.ascontiguousarray(x, dtype=np.float32)
    params = np.asarray(params, f8s)
    rot = params[:, :D]
    ent = params[:, D : 2 * D]
    scale = 1.0 / np.sqrt(D)
    Wqp = (np.asarray(Wq, f8s) @ rot) * scale
    Wkp = np.asarray(Wk, f8s) @ ent
    bqs = np.asarray(bq, f8s) * scale
    G = Wkp.T @ Wqp
    w = Wkp.T @ bqs
    gmat = np.ascontiguousarray(
        np.vstack([G.T, w[None, :]]).astype(np.float32))
    wvb = np.ascontiguousarray(
        np.vstack([np.asarray(Wv, np.float32).T,
                   np.asarray(bv, np.float32)[None]]))
    ident = np.eye(128, dtype=np.float32)
    return x, gmat, wvb, ident


def kernel(x, params, Wq, bq, Wk, bk, Wv, bv, _trace=False):
    x, gmat, wvb, ident = _prep_inputs(x, params, Wq, bq, Wk, bk, Wv, bv)
    if "nc" not in _CACHED:
        _CACHED["nc"] = build_kernel()
    nc = _CACHED["nc"]
    in_maps = []
    for c in range(N_CORES):
        in_maps.append({
            "x": x,
            "xq": np.ascontiguousarray(x[c * QR : (c + 1) * QR]),
            "gmat": gmat, "wvb": wvb, "ident": ident, "nident": -ident,
            "onescol": np.ones([128, 128], np.float32),
        })
    res = run_bass_kernel_spmd(nc, in_maps, core_ids=list(range(N_CORES)),
                               trace=_trace)
    out = np.concatenate([res.results[c]["y"] for c in range(N_CORES)], axis=0)
    global _CACHED_RES
    _CACHED_RES = res
    return out


# revision 40
# speedup vs baseline: 1.0047x; 1.0047x over previous
"""Classical self-attention on 8 trn2 NeuronCores — v3.

N=16384 tokens, d=64, fp32. Sequence-parallel over Q: core c handles rows
[c*2048, (c+1)*2048). Per-core differentiation via the xq input slice.

Algebra (host-folded):
  s_jq = k_j . q_q = x_j^T (G x_q + w) + const_q, G/w precomputed on host;
  the per-q const is absorbed by the softmax shift, so it is never computed.
  sum_j p_j v_j = [Wv|bv] @ (sum_j p_j [x_j;1])  -> V never materialized;
  the ones column of xh doubles as the denominator row.

Structure per core:
  setup: DMA x (8 chunks) + xq; PE-transpose x -> xT[65,16384] (ones row);
    fp8 shadows x8[32,2,N] (DoubleRow d-split: d = slot*32 + partition);
    g-proj gB[65,2048] = [G^T;w^T] @ xqT, fp8 shadow g8; pass A for group 0.
  pass A (row max): fp8 DR matmuls psA[128,1024] per (tile, 1024-key chunk);
    DVE tensor_tensor_reduce (max,max) -> mms[:,t,c]; mfin: reduce -> mt,
    PE transpose -> gB row 64 = -m̂.
  8 phases (groups of 256 q = 2 tiles): per unit (4 kv blocks):
    PV(u-2) x4, B x4 (fp32r, psB[:,u%2,:]), exp(u-1) on ACT, TTR(u-1),
    A-mm(u) for group h+1 last (spacing hides the single-buffered psA WAR).
  tail: psPV[65,256] -> OT; after phase 7: OTw = [Wv|bv]^T @ OT (+denom row),
    PE transpose, DVE reciprocal+scale, DMA out.
"""

import sys

sys.path.insert(0, "/opt/trn_rl_repo")

from contextlib import ExitStack

import numpy as np

import concourse.bass as bass
import concourse.mybir as mybir
import concourse.tile as tile
from concourse import bacc
from concourse.bass import ds, ts
from concourse.bass_utils import run_bass_kernel_spmd

N_CORES = 8
N = 16384
D = 64
QR = N // N_CORES          # 2048 q rows per core
N_QTILE = QR // 128        # 16 q tiles per core
N_PHASE = 8                # groups of 2 q-tiles (256 q cols)
N_KV_BLK = N // 128        # 128 kv blocks
N_UNIT = 32                # units per phase (4 blocks each)
N_ACH = 16                 # 1024-key A-chunks per q tile
F32 = mybir.dt.float32
F8 = mybir.dt.float8e4
R32 = mybir.dt.float32r
DR = mybir.MatmulPerfMode.DoubleRow
AX = mybir.AxisListType.X
MAX = mybir.AluOpType.max

_CACHED = {}


def build_kernel():
    nc = bacc.Bacc("TRN2", target_bir_lowering=False, debug=False,
                   num_devices=N_CORES)

    x_d = nc.dram_tensor("x", [N, D], F32, kind="ExternalInput")
    xq_d = nc.dram_tensor("xq", [QR, D], F32, kind="ExternalInput")
    gm_d = nc.dram_tensor("gmat", [D + 1, D], F32, kind="ExternalInput")
    wv_d = nc.dram_tensor("wvb", [D + 1, D], F32, kind="ExternalInput")
    id_d = nc.dram_tensor("ident", [128, 128], F32, kind="ExternalInput")
    nid_d = nc.dram_tensor("nident", [128, 128], F32, kind="ExternalInput")
    oc_d = nc.dram_tensor("onescol", [128, 128], F32, kind="ExternalInput")
    y_d = nc.dram_tensor("y", [QR, D], F32, kind="ExternalOutput")

    def rb(ap):
        return ap.bitcast(R32)

    with tile.TileContext(nc) as tc, ExitStack() as ctx:
        sb = ctx.enter_context(tc.tile_pool(name="sb", bufs=1))
        expp = ctx.enter_context(tc.tile_pool(name="expp", bufs=4))
        smp = ctx.enter_context(tc.tile_pool(name="smp", bufs=4))
        scrp = ctx.enter_context(tc.tile_pool(name="scr", bufs=2))
        psB_p = ctx.enter_context(tc.tile_pool(name="psB", bufs=1, space="PSUM"))
        psA_p = ctx.enter_context(tc.tile_pool(name="psA", bufs=1, space="PSUM"))
        psPV_p = ctx.enter_context(tc.tile_pool(name="psPV", bufs=1, space="PSUM"))
        psM_p = ctx.enter_context(tc.tile_pool(name="psM", bufs=1, space="PSUM"))

        # ---- persistent SBUF ----
        xT = sb.tile([D + 1, N], F32)          # x^T, row 64 = ones
        x8 = sb.tile([32, 2, N], F8)           # DR d-split shadow of xT
        xV = sb.tile([128, N_KV_BLK, D + 1], F32)  # natural x, col 64 = ones
        xqn = sb.tile([128, N_QTILE, D + 1], F32)  # natural xq + ones col
        xqT = sb.tile([D + 1, QR], F32)        # xq^T, row 64 = ones
        gB = sb.tile([D + 1, QR], F32)         # g, row 64 = -m̂
        g8 = sb.tile([32, 2, QR], F8)
        gmat = sb.tile([D + 1, D], F32)
        wvb = sb.tile([D + 1, D], F32)
        ident = sb.tile([128, 128], F32)
        nident = sb.tile([128, 128], F32)
        onesb = sb.tile([128, 128], F32)
        mms = sb.tile([128, N_QTILE, 24], F32)
        # OT rows 0:64: out1 = sum p x, overwritten in-place by the
        # [Wv|bv]-stage at each phase tail; row 64 = denominator (sum p).
        OT = sb.tile([D + 1, QR], F32)
        y_sb = sb.tile([128, N_QTILE, D], F32)

        # ---- PSUM ----
        # The tile framework tracks PSUM hazards at tile granularity (and
        # marks PSUM-reading ACT/DVE ops as writers), so double-buffering
        # must use SEPARATE tiles, not halves of one tile.
        psBs = [psB_p.tile([128, 1024], F32, tag="psba", name="psba"),
                psB_p.tile([128, 1024], F32, tag="psbb", name="psbb")]
        psAb = psA_p.tile([128, 1024], F32, tag="psab", name="psab")
        psAsm = psA_p.tile([128, 512], F32, tag="psas", name="psas")
        # psPV bank also hosts pneg (partition 127) - disjoint regions of
        # one tile; tile-granular hazards only serialize phase tails.
        psPV = psPV_p.tile([128, 512], F32)

        # ---- DMAs (DMA_ENGINES serializes; order by first use) ----
        nc.gpsimd.dma_start(ident[:], id_d[:])
        nc.gpsimd.dma_start(gmat[:].bitcast(R32), gm_d[:].bitcast(R32))
        nc.gpsimd.dma_start(wvb[:], wv_d[:])
        nc.gpsimd.dma_start(nident[:], nid_d[:])
        nc.gpsimd.dma_start(onesb[:], oc_d[:])
        nc.sync.dma_start(
            xqn[:, :, 0:D], xq_d[:].rearrange("(j p) d -> p j d", p=128))
        nc.gpsimd.memset(xqn[:, :, D : D + 1], 1.0)
        for c in range(8):
            nc.sync.dma_start(
                xV[:, ds(c * 16, 16), 0:D].bitcast(R32),
                x_d[ds(c * 2048, 2048), :].bitcast(R32).rearrange(
                    "(j p) d -> p j d", p=128))
        # r32 memset fails the ISA check; copy a DMA-loaded ones tile
        nc.vector.tensor_copy(rb(xV[:, :, D : D + 1]), onesb[:, :].rearrange("p (j o) -> p j o", o=1))

        # ---- xq transposes (borrow psB halves) + g-proj + g8 ----
        for o in range(2):
            tgt = psBs[o % 2]
            for j in range(8):
                nc.tensor.transpose(tgt[0 : D + 1, ts(j, 128)],
                                    xqn[:, o * 8 + j, :], ident[:])
            nc.vector.tensor_copy(rb(xqT[:, ts(o, 1024)]), tgt[0 : D + 1, :])
        for s in range(8):
            pm = psAb[0:D, ds((s % 2) * 256, 256)]
            nc.tensor.matmul(pm, rb(gmat[:]), rb(xqT[:, ts(s, 256)]),
                             start=True, stop=True)
            nc.vector.tensor_copy(rb(gB[0:D, ts(s, 256)]), pm)
        nc.scalar.copy(g8[:, 0, :], gB[0:32, :])
        nc.scalar.copy(g8[:, 1, :], gB[32:64, :])

        # ---- pass-A plumbing ----
        # DVE must scan all N keys per q tile (hw: one PSUM operand per DVE
        # op, so no pairing); big 1024-key chunks amortize access overhead,
        # a second 512-key tile keeps the psA WAR chains off the PE queue.
        # Even q tiles take big chunks from keys [0,8K), odd from [8K,16K).
        def a_off(t, big):
            return (0 if t % 2 == 0 else 8192) if big else                    (8192 if t % 2 == 0 else 0)

        def emit_A_big(t, c):
            # matmul psum output is limited to one bank (512 fp32): two
            # half-matmuls fill the tile, one 1024-wide reduce drains it.
            for h in range(2):
                nc.tensor.matmul(
                    psAb[:, ds(h * 512, 512)], g8[:, :, ts(t, 128)],
                    x8[:, :, ds(a_off(t, True) + c * 1024 + h * 512, 512)],
                    start=True, stop=True, perf_mode=DR)

        def emit_red_big(t, c):
            nc.vector.reduce_max(mms[:, t, c : c + 1], psAb[:], axis=AX)

        def emit_A_sm(t, j):
            nc.tensor.matmul(psAsm[:], g8[:, :, ts(t, 128)],
                             x8[:, :, ds(a_off(t, False) + j * 512, 512)],
                             start=True, stop=True, perf_mode=DR)

        def emit_red_sm(t, j):
            nc.vector.reduce_max(mms[:, t, 8 + j : 9 + j], psAsm[:], axis=AX)

        def emit_mfin(t):
            mt = smp.tile([128, 1], F32, tag="mt")
            nc.vector.reduce_max(mt[:], mms[:, t, :], axis=AX)
            # pneg = mt^T @ (-I) = -m̂ row; plain DVE copy into gB row 64
            # keeps the hot ACT queue out of the phase-boundary chain.
            pneg = psPV[64:65, 256:384]
            nc.tensor.matmul(pneg, mt[:], nident[:], start=True, stop=True)
            nc.vector.tensor_copy(rb(gB[D : D + 1, ts(t, 128)]), pneg)

        # ---- setup: kv chunks -> xT, x8, pass A for group 0 ----
        for c in range(8):
            for oo in range(2):
                o = 2 * c + oo
                tgt = psBs[o % 2]
                for j in range(8):
                    nc.tensor.transpose(tgt[0 : D + 1, ts(j, 128)],
                                        xV[:, o * 8 + j, :], ident[:])
                nc.scalar.copy(rb(xT[:, ts(o, 1024)]), tgt[0 : D + 1, :])
                nc.scalar.copy(x8[:, 0, ts(o, 1024)], tgt[0:32, :])
                # Pool cannot read PSUM: slot-1 shadow reads xT after its copy
                nc.gpsimd.tensor_copy(x8[:, 1, ts(o, 1024)],
                                      xT[32:64, ts(o, 1024)])
                # group-0 A work: oct o = keys [1024*o, 1024*o+1024).
                # tile 0 (even): big for o<8, sm pairs for o>=8; tile 1
                # (odd): the reverse. One big + two sm per oct.
                if o < 8:
                    emit_A_big(0, o)
                    emit_red_big(0, o)
                    emit_A_sm(1, 2 * o)
                    emit_red_sm(1, 2 * o)
                    emit_A_sm(1, 2 * o + 1)
                    emit_red_sm(1, 2 * o + 1)
                else:
                    emit_A_big(1, o - 8)
                    emit_red_big(1, o - 8)
                    emit_A_sm(0, 2 * (o - 8))
                    emit_red_sm(0, 2 * (o - 8))
                    emit_A_sm(0, 2 * (o - 8) + 1)
                    emit_red_sm(0, 2 * (o - 8) + 1)
        emit_mfin(0)
        emit_mfin(1)

        # ---- phases ----
        for h in range(N_PHASE):
            qsl = ds(h * 256, 256)
            atiles = (2 * (h + 1), 2 * (h + 1) + 1) if h < N_PHASE - 1 else None
            pv_q = []   # batches of (blk, ex_ap); popped two units later
            # A-work queue: per tile 8 big + 16 sm ops; interleave
            # big/sm so the single-buffered tiles get WAR slack. Front-load
            # slightly so mfin chains clear before the next phase starts.
            a_q = []
            if atiles is not None:
                for t in atiles:
                    for c in range(8):
                        a_q.append(("big", t, c))
                        a_q.append(("sm", t, 2 * c))
                        a_q.append(("sm", t, 2 * c + 1))
            a_done = {}

            def emit_exp(u):
                ex = expp.tile([128, 1024], F32, tag="ex")
                nc.scalar.activation(rb(ex[:]), psBs[u % 2][:],
                                     mybir.ActivationFunctionType.Exp)
                pv_q.append([(4 * u + j, ex[:, ds(j * 256, 256)])
                             for j in range(4)])

            def emit_pv():
                for blk, ex_ap in pv_q.pop(0):
                    nc.tensor.matmul(psPV[0 : D + 1, 0:256], rb(xV[:, blk, :]),
                                     rb(ex_ap), start=(blk == 0),
                                     stop=(blk == N_KV_BLK - 1),
                                     skip_group_check=True)

            for u in range(N_UNIT):
                # Bs first: they feed this unit's exp with no other deps.
                for j in range(4):
                    blk = 4 * u + j
                    nc.tensor.matmul(psBs[u % 2][:, ds(j * 256, 256)],
                                     rb(xT[:, ts(blk, 128)]), rb(gB[:, qsl]),
                                     start=True, stop=True)
                # PVs of unit u-2: their exp finished during unit u-1.
                if len(pv_q) >= 2:
                    emit_pv()
                n_a = 2 if u < 8 else (2 if u % 2 == 0 else 1)
                for _ in range(min(n_a, len(a_q))):
                    kind, t, k = a_q.pop(0)
                    if kind == "big":
                        emit_A_big(t, k)
                        emit_red_big(t, k)
                    else:
                        emit_A_sm(t, k)
                        emit_red_sm(t, k)
                    a_done[t] = a_done.get(t, 0) + 1
                    if a_done[t] == 24:
                        emit_mfin(t)
                emit_exp(u)
            # tail: mfin chains are already emitted; flush PVs, save out1
            while pv_q:
                emit_pv()
            nc.vector.tensor_copy(rb(OT[:, qsl]), psPV[0 : D + 1, 0:256])
            # [Wv|bv]-stage for this group, written back into OT rows 0:64;
            # then the denominator row becomes its reciprocal in place, so
            # the final transpose carries 1/denom in column 64. Exact fp32:
            # 8 matmuls of 256 cols, precision is worth the 4x row cost.
            pm = psAb[0:D, 256:512]
            nc.tensor.matmul(pm, wvb[:], OT[:, qsl],
                             start=True, stop=True)
            nc.vector.tensor_copy(rb(OT[0:D, qsl]), pm)
            # reciprocal can't emit fp32r; bounce through scratch + rb copy
            rrow = scrp.tile([128, 256], F32, tag="scr")
            nc.vector.reciprocal(rrow[0:1, :], OT[D : D + 1, qsl])
            nc.vector.tensor_copy(rb(OT[D : D + 1, qsl]), rrow[0:1, :])

        # ---- final: transpose OT (col 64 = 1/denom), normalize, store ----
        for t in range(N_QTILE):
            pO = psBs[t % 2][:, 0 : D + 1]
            nc.tensor.transpose(pO, OT[:, ts(t, 128)],
                                ident[0 : D + 1, 0 : D + 1])
            nc.vector.tensor_scalar_mul(y_sb[:, t, :], pO[:, 0:D],
                                        pO[:, D : D + 1])
            if t % 4 == 3:
                nc.sync.dma_start(
                    y_d[ds((t - 3) * 128, 512), :].rearrange(
                        "(t p) d -> p t d", p=128),
                    y_sb[:, ds(t - 3, 4), :])

    nc.compile()
    return nc


def _prep_inputs(x, params, Wq, bq, Wk, bk, Wv, bv):
    f8s = np.float64
    x = np.ascontiguousarray(x, dtype=np.float32)
    params = np.asarray(params, f8s)
    rot = params[:, :D]
    ent = params[:, D : 2 * D]
    scale = 1.0 / np.sqrt(D)
    Wqp = (np.asarray(Wq, f8s) @ rot) * scale
    Wkp = np.asarray(Wk, f8s) @ ent
    bqs = np.asarray(bq, f8s) * scale
    G = Wkp.T @ Wqp
    w = Wkp.T @ bqs
    gmat = np.ascontiguousarray(
        np.vstack([G.T, w[None, :]]).astype(np.float32))
    wvb = np.ascontiguousarray(
        np.vstack([np.asarray(Wv, np.float32).T,
                   np.asarray(bv, np.float32)[None]]))
    ident = np.eye(128, dtype=np.float32)
    return x, gmat, wvb, ident


def kernel(x, params, Wq, bq, Wk, bk, Wv, bv, _trace=False):
    x, gmat, wvb, ident = _prep_inputs(x, params, Wq, bq, Wk, bk, Wv, bv)
    if "nc" not in _CACHED:
        _CACHED["nc"] = build_kernel()
    nc = _CACHED["nc"]
    in_maps = []
    for c in range(N_CORES):
        in_maps.append({
            "x": x,
            "xq": np.ascontiguousarray(x[c * QR : (c + 1) * QR]),
            "gmat": gmat, "wvb": wvb, "ident": ident, "nident": -ident,
            "onescol": np.ones([128, 128], np.float32),
        })
    res = run_bass_kernel_spmd(nc, in_maps, core_ids=list(range(N_CORES)),
                               trace=_trace)
    out = np.concatenate([res.results[c]["y"] for c in range(N_CORES)], axis=0)
    global _CACHED_RES
    _CACHED_RES = res
    return out


# revision 41
# speedup vs baseline: 1.0119x; 1.0072x over previous
"""Classical self-attention on 8 trn2 NeuronCores — v3.

N=16384 tokens, d=64, fp32. Sequence-parallel over Q: core c handles rows
[c*2048, (c+1)*2048). Per-core differentiation via the xq input slice.

Algebra (host-folded):
  s_jq = k_j . q_q = x_j^T (G x_q + w) + const_q, G/w precomputed on host;
  the per-q const is absorbed by the softmax shift, so it is never computed.
  sum_j p_j v_j = [Wv|bv] @ (sum_j p_j [x_j;1])  -> V never materialized;
  the ones column of xh doubles as the denominator row.

Structure per core:
  setup: DMA x (8 chunks) + xq; PE-transpose x -> xT[65,16384] (ones row);
    fp8 shadows x8[32,2,N] (DoubleRow d-split: d = slot*32 + partition);
    g-proj gB[65,2048] = [G^T;w^T] @ xqT, fp8 shadow g8; pass A for group 0.
  pass A (row max): fp8 DR matmuls psA[128,1024] per (tile, 1024-key chunk);
    DVE tensor_tensor_reduce (max,max) -> mms[:,t,c]; mfin: reduce -> mt,
    PE transpose -> gB row 64 = -m̂.
  8 phases (groups of 256 q = 2 tiles): per unit (4 kv blocks):
    PV(u-2) x4, B x4 (fp32r, psB[:,u%2,:]), exp(u-1) on ACT, TTR(u-1),
    A-mm(u) for group h+1 last (spacing hides the single-buffered psA WAR).
  tail: psPV[65,256] -> OT; after phase 7: OTw = [Wv|bv]^T @ OT (+denom row),
    PE transpose, DVE reciprocal+scale, DMA out.
"""

import sys

sys.path.insert(0, "/opt/trn_rl_repo")

from contextlib import ExitStack

import numpy as np

import concourse.bass as bass
import concourse.mybir as mybir
import concourse.tile as tile
from concourse import bacc
from concourse.bass import ds, ts
from concourse.bass_utils import run_bass_kernel_spmd

N_CORES = 8
N = 16384
D = 64
QR = N // N_CORES          # 2048 q rows per core
N_QTILE = QR // 128        # 16 q tiles per core
N_PHASE = 8                # groups of 2 q-tiles (256 q cols)
N_KV_BLK = N // 128        # 128 kv blocks
N_UNIT = 32                # units per phase (4 blocks each)
N_ACH = 16                 # 1024-key A-chunks per q tile
F32 = mybir.dt.float32
F8 = mybir.dt.float8e4
R32 = mybir.dt.float32r
DR = mybir.MatmulPerfMode.DoubleRow
AX = mybir.AxisListType.X
MAX = mybir.AluOpType.max

_CACHED = {}


def build_kernel():
    nc = bacc.Bacc("TRN2", target_bir_lowering=False, debug=False,
                   num_devices=N_CORES)

    x_d = nc.dram_tensor("x", [N, D], F32, kind="ExternalInput")
    xq_d = nc.dram_tensor("xq", [QR, D], F32, kind="ExternalInput")
    gm_d = nc.dram_tensor("gmat", [D + 1, D], F32, kind="ExternalInput")
    wv_d = nc.dram_tensor("wvb", [D + 1, D], F32, kind="ExternalInput")
    id_d = nc.dram_tensor("ident", [128, 128], F32, kind="ExternalInput")
    nid_d = nc.dram_tensor("nident", [128, 128], F32, kind="ExternalInput")
    oc_d = nc.dram_tensor("onescol", [128, 128], F32, kind="ExternalInput")
    y_d = nc.dram_tensor("y", [QR, D], F32, kind="ExternalOutput")

    def rb(ap):
        return ap.bitcast(R32)

    with tile.TileContext(nc) as tc, ExitStack() as ctx:
        sb = ctx.enter_context(tc.tile_pool(name="sb", bufs=1))
        expp = ctx.enter_context(tc.tile_pool(name="expp", bufs=4))
        smp = ctx.enter_context(tc.tile_pool(name="smp", bufs=4))
        scrp = ctx.enter_context(tc.tile_pool(name="scr", bufs=2))
        psB_p = ctx.enter_context(tc.tile_pool(name="psB", bufs=1, space="PSUM"))
        psA_p = ctx.enter_context(tc.tile_pool(name="psA", bufs=1, space="PSUM"))
        psPV_p = ctx.enter_context(tc.tile_pool(name="psPV", bufs=1, space="PSUM"))
        psM_p = ctx.enter_context(tc.tile_pool(name="psM", bufs=1, space="PSUM"))

        # ---- persistent SBUF ----
        xT = sb.tile([D + 1, N], F32)          # x^T, row 64 = ones
        x8 = sb.tile([32, 2, N], F8)           # DR d-split shadow of xT
        xV = sb.tile([128, N_KV_BLK, D + 1], F32)  # natural x, col 64 = ones
        xqn = sb.tile([128, N_QTILE, D + 1], F32)  # natural xq + ones col
        xqT = sb.tile([D + 1, QR], F32)        # xq^T, row 64 = ones
        gB = sb.tile([D + 1, QR], F32)         # g, row 64 = -m̂
        g8 = sb.tile([32, 2, QR], F8)
        gmat = sb.tile([D + 1, D], F32)
        wvb = sb.tile([D + 1, D], F32)
        ident = sb.tile([128, 128], F32)
        nident = sb.tile([128, 128], F32)
        onesb = sb.tile([128, 128], F32)
        mms = sb.tile([128, N_QTILE, 24], F32)
        # OT rows 0:64: out1 = sum p x, overwritten in-place by the
        # [Wv|bv]-stage at each phase tail; row 64 = denominator (sum p).
        OT = sb.tile([D + 1, QR], F32)
        y_sb = sb.tile([128, N_QTILE, D], F32)

        # ---- PSUM ----
        # The tile framework tracks PSUM hazards at tile granularity (and
        # marks PSUM-reading ACT/DVE ops as writers), so double-buffering
        # must use SEPARATE tiles, not halves of one tile.
        psBs = [psB_p.tile([128, 1024], F32, tag="psba", name="psba"),
                psB_p.tile([128, 1024], F32, tag="psbb", name="psbb")]
        psAb = psA_p.tile([128, 1024], F32, tag="psab", name="psab")
        psAsm = psA_p.tile([128, 512], F32, tag="psas", name="psas")
        # psPV bank also hosts pneg (partition 127) - disjoint regions of
        # one tile; tile-granular hazards only serialize phase tails.
        psPV = psPV_p.tile([128, 512], F32)

        # ---- DMAs (DMA_ENGINES serializes; order by first use) ----
        nc.gpsimd.dma_start(ident[:], id_d[:])
        nc.gpsimd.dma_start(gmat[:].bitcast(R32), gm_d[:].bitcast(R32))
        nc.gpsimd.dma_start(wvb[:], wv_d[:])
        nc.gpsimd.dma_start(nident[:], nid_d[:])
        nc.gpsimd.dma_start(onesb[:], oc_d[:])
        nc.sync.dma_start(
            xqn[:, :, 0:D], xq_d[:].rearrange("(j p) d -> p j d", p=128))
        nc.gpsimd.memset(xqn[:, :, D : D + 1], 1.0)
        for c in range(8):
            nc.sync.dma_start(
                xV[:, ds(c * 16, 16), 0:D].bitcast(R32),
                x_d[ds(c * 2048, 2048), :].bitcast(R32).rearrange(
                    "(j p) d -> p j d", p=128))
        # r32 memset fails the ISA check; copy a DMA-loaded ones tile
        nc.vector.tensor_copy(rb(xV[:, :, D : D + 1]), onesb[:, :].rearrange("p (j o) -> p j o", o=1))

        # ---- xq transposes (borrow psB halves) + g-proj + g8 ----
        for o in range(2):
            tgt = psBs[o % 2]
            for j in range(8):
                nc.tensor.transpose(tgt[0 : D + 1, ts(j, 128)],
                                    xqn[:, o * 8 + j, :], ident[:])
            nc.vector.tensor_copy(rb(xqT[:, ts(o, 1024)]), tgt[0 : D + 1, :])
        for s in range(8):
            pm = psAb[0:D, ds((s % 2) * 256, 256)]
            nc.tensor.matmul(pm, rb(gmat[:]), rb(xqT[:, ts(s, 256)]),
                             start=True, stop=True)
            nc.vector.tensor_copy(rb(gB[0:D, ts(s, 256)]), pm)
        nc.scalar.copy(g8[:, 0, :], gB[0:32, :])
        nc.scalar.copy(g8[:, 1, :], gB[32:64, :])

        # ---- pass-A plumbing ----
        # DVE must scan all N keys per q tile (hw: one PSUM operand per DVE
        # op, so no pairing); big 1024-key chunks amortize access overhead,
        # a second 512-key tile keeps the psA WAR chains off the PE queue.
        # Even q tiles take big chunks from keys [0,8K), odd from [8K,16K).
        def a_off(t, big):
            return (0 if t % 2 == 0 else 8192) if big else                    (8192 if t % 2 == 0 else 0)

        def emit_A_big(t, c):
            # matmul psum output is limited to one bank (512 fp32): two
            # half-matmuls fill the tile, one 1024-wide reduce drains it.
            for h in range(2):
                nc.tensor.matmul(
                    psAb[:, ds(h * 512, 512)], g8[:, :, ts(t, 128)],
                    x8[:, :, ds(a_off(t, True) + c * 1024 + h * 512, 512)],
                    start=True, stop=True, perf_mode=DR)

        def emit_red_big(t, c):
            nc.vector.reduce_max(mms[:, t, c : c + 1], psAb[:], axis=AX)

        def emit_A_sm(t, j):
            nc.tensor.matmul(psAsm[:], g8[:, :, ts(t, 128)],
                             x8[:, :, ds(a_off(t, False) + j * 512, 512)],
                             start=True, stop=True, perf_mode=DR)

        def emit_red_sm(t, j):
            nc.vector.reduce_max(mms[:, t, 8 + j : 9 + j], psAsm[:], axis=AX)

        def emit_mfin(t):
            mt = smp.tile([128, 1], F32, tag="mt")
            nc.vector.reduce_max(mt[:], mms[:, t, :], axis=AX)
            # pneg = mt^T @ (-I) = -m̂ row; plain DVE copy into gB row 64
            # keeps the hot ACT queue out of the phase-boundary chain.
            pneg = psPV[64:65, 256:384]
            nc.tensor.matmul(pneg, mt[:], nident[:], start=True, stop=True)
            nc.vector.tensor_copy(rb(gB[D : D + 1, ts(t, 128)]), pneg)

        # ---- setup: kv chunks -> xT, x8, pass A for group 0 ----
        for c in range(8):
            for oo in range(2):
                o = 2 * c + oo
                tgt = psBs[o % 2]
                for j in range(8):
                    nc.tensor.transpose(tgt[0 : D + 1, ts(j, 128)],
                                        xV[:, o * 8 + j, :], ident[:])
                nc.scalar.copy(rb(xT[:, ts(o, 1024)]), tgt[0 : D + 1, :])
                nc.scalar.copy(x8[:, 0, ts(o, 1024)], tgt[0:32, :])
                # Pool cannot read PSUM: slot-1 shadow reads xT after its copy
                nc.gpsimd.tensor_copy(x8[:, 1, ts(o, 1024)],
                                      xT[32:64, ts(o, 1024)])
                # group-0 A work: oct o = keys [1024*o, 1024*o+1024).
                # tile 0 (even): big for o<8, sm pairs for o>=8; tile 1
                # (odd): the reverse. One big + two sm per oct.
                if o < 8:
                    emit_A_big(0, o)
                    emit_red_big(0, o)
                    emit_A_sm(1, 2 * o)
                    emit_red_sm(1, 2 * o)
                    emit_A_sm(1, 2 * o + 1)
                    emit_red_sm(1, 2 * o + 1)
                else:
                    emit_A_big(1, o - 8)
                    emit_red_big(1, o - 8)
                    emit_A_sm(0, 2 * (o - 8))
                    emit_red_sm(0, 2 * (o - 8))
                    emit_A_sm(0, 2 * (o - 8) + 1)
                    emit_red_sm(0, 2 * (o - 8) + 1)
        emit_mfin(0)
        emit_mfin(1)

        # ---- phases ----
        for h in range(N_PHASE):
            qsl = ds(h * 256, 256)
            atiles = (2 * (h + 1), 2 * (h + 1) + 1) if h < N_PHASE - 1 else None
            pv_q = []   # batches of (blk, ex_ap); popped two units later
            # A-work queue: per tile 8 big + 16 sm ops; interleave
            # big/sm so the single-buffered tiles get WAR slack. Front-load
            # slightly so mfin chains clear before the next phase starts.
            a_q = []
            if atiles is not None:
                for t in atiles:
                    for c in range(8):
                        a_q.append(("sm", t, 2 * c))
                        a_q.append(("big", t, c))
                        a_q.append(("sm", t, 2 * c + 1))
            a_done = {}

            def emit_exp(u):
                ex = expp.tile([128, 1024], F32, tag="ex")
                nc.scalar.activation(rb(ex[:]), psBs[u % 2][:],
                                     mybir.ActivationFunctionType.Exp)
                pv_q.append([(4 * u + j, ex[:, ds(j * 256, 256)])
                             for j in range(4)])

            def emit_pv():
                for blk, ex_ap in pv_q.pop(0):
                    nc.tensor.matmul(psPV[0 : D + 1, 0:256], rb(xV[:, blk, :]),
                                     rb(ex_ap), start=(blk == 0),
                                     stop=(blk == N_KV_BLK - 1),
                                     skip_group_check=True)

            for u in range(N_UNIT):
                # Bs first: they feed this unit's exp with no other deps.
                for j in range(4):
                    blk = 4 * u + j
                    nc.tensor.matmul(psBs[u % 2][:, ds(j * 256, 256)],
                                     rb(xT[:, ts(blk, 128)]), rb(gB[:, qsl]),
                                     start=True, stop=True)
                # PVs of unit u-2: their exp finished during unit u-1.
                if len(pv_q) >= 2:
                    emit_pv()
                n_a = 0 if u == 0 else (2 if (u % 2 == 1 or u < 12) else 1)
                for _ in range(min(n_a, len(a_q))):
                    kind, t, k = a_q.pop(0)
                    if kind == "big":
                        emit_A_big(t, k)
                        emit_red_big(t, k)
                    else:
                        emit_A_sm(t, k)
                        emit_red_sm(t, k)
                    a_done[t] = a_done.get(t, 0) + 1
                    if a_done[t] == 24:
                        emit_mfin(t)
                emit_exp(u)
            # tail: mfin chains are already emitted; flush PVs, save out1
            while pv_q:
                emit_pv()
            nc.scalar.copy(rb(OT[:, qsl]), psPV[0 : D + 1, 0:256])
            # [Wv|bv]-stage for this group, written back into OT rows 0:64;
            # then the denominator row becomes its reciprocal in place, so
            # the final transpose carries 1/denom in column 64. Exact fp32:
            # 8 matmuls of 256 cols, precision is worth the 4x row cost.
            pm = psAb[0:D, 256:512]
            nc.tensor.matmul(pm, wvb[:], OT[:, qsl],
                             start=True, stop=True)
            nc.scalar.copy(rb(OT[0:D, qsl]), pm)
            # reciprocal can't emit fp32r; bounce through scratch + rb copy
            rrow = scrp.tile([128, 256], F32, tag="scr")
            nc.vector.reciprocal(rrow[0:1, :], OT[D : D + 1, qsl])
            nc.vector.tensor_copy(rb(OT[D : D + 1, qsl]), rrow[0:1, :])

        # ---- final: transpose OT (col 64 = 1/denom), normalize, store ----
        for t in range(N_QTILE):
            pO = psBs[t % 2][:, 0 : D + 1]
            nc.tensor.transpose(pO, OT[:, ts(t, 128)],
                                ident[0 : D + 1, 0 : D + 1])
            nc.vector.tensor_scalar_mul(y_sb[:, t, :], pO[:, 0:D],
                                        pO[:, D : D + 1])
            if t % 4 == 3:
                nc.sync.dma_start(
                    y_d[ds((t - 3) * 128, 512), :].rearrange(
                        "(t p) d -> p t d", p=128),
                    y_sb[:, ds(t - 3, 4), :])

    nc.compile()
    return nc


def _prep_inputs(x, params, Wq, bq, Wk, bk, Wv, bv):
    f8s = np.float64
    x = np.ascontiguousarray(x, dtype=np.float32)
    params = np.asarray(params, f8s)
    rot = params[:, :D]
    ent = params[:, D : 2 * D]
    scale = 1.0 / np.sqrt(D)
    Wqp = (np.asarray(Wq, f8s) @ rot) * scale
    Wkp = np.asarray(Wk, f8s) @ ent
    bqs = np.asarray(bq, f8s) * scale
    G = Wkp.T @ Wqp
    w = Wkp.T @ bqs
    gmat = np.ascontiguousarray(
        np.vstack([G.T, w[None, :]]).astype(np.float32))
    wvb = np.ascontiguousarray(
        np.vstack([np.asarray(Wv, np.float32).T,
                   np.asarray(bv, np.float32)[None]]))
    ident = np.eye(128, dtype=np.float32)
    return x, gmat, wvb, ident


def kernel(x, params, Wq, bq, Wk, bk, Wv, bv, _trace=False):
    x, gmat, wvb, ident = _prep_inputs(x, params, Wq, bq, Wk, bk, Wv, bv)
    if "nc" not in _CACHED:
        _CACHED["nc"] = build_kernel()
    nc = _CACHED["nc"]
    in_maps = []
    for c in range(N_CORES):
        in_maps.append({
            "x": x,
            "xq": np.ascontiguousarray(x[c * QR : (c + 1) * QR]),
            "gmat": gmat, "wvb": wvb, "ident": ident, "nident": -ident,
            "onescol": np.ones([128, 128], np.float32),
        })
    res = run_bass_kernel_spmd(nc, in_maps, core_ids=list(range(N_CORES)),
                               trace=_trace)
    out = np.concatenate([res.results[c]["y"] for c in range(N_CORES)], axis=0)
    global _CACHED_RES
    _CACHED_RES = res
    return out


# revision 43
# speedup vs baseline: 1.1103x; 1.0972x over previous
"""Classical self-attention on 8 trn2 NeuronCores — v3.

N=16384 tokens, d=64, fp32. Sequence-parallel over Q: core c handles rows
[c*2048, (c+1)*2048). Per-core differentiation via the xq input slice.

Algebra (host-folded):
  s_jq = k_j . q_q = x_j^T (G x_q + w) + const_q, G/w precomputed on host;
  the per-q const is absorbed by the softmax shift, so it is never computed.
  sum_j p_j v_j = [Wv|bv] @ (sum_j p_j [x_j;1])  -> V never materialized;
  the ones column of xh doubles as the denominator row.

Structure per core:
  setup: DMA x (8 chunks) + xq; PE-transpose x -> xT[65,16384] (ones row);
    fp8 shadows x8[32,2,N] (DoubleRow d-split: d = slot*32 + partition);
    g-proj gB[65,2048] = [G^T;w^T] @ xqT, fp8 shadow g8; pass A for group 0.
  pass A (row max): fp8 DR matmuls psA[128,1024] per (tile, 1024-key chunk);
    DVE tensor_tensor_reduce (max,max) -> mms[:,t,c]; mfin: reduce -> mt,
    PE transpose -> gB row 64 = -m̂.
  8 phases (groups of 256 q = 2 tiles): per unit (4 kv blocks):
    PV(u-2) x4, B x4 (fp32r, psB[:,u%2,:]), exp(u-1) on ACT, TTR(u-1),
    A-mm(u) for group h+1 last (spacing hides the single-buffered psA WAR).
  tail: psPV[65,256] -> OT; after phase 7: OTw = [Wv|bv]^T @ OT (+denom row),
    PE transpose, DVE reciprocal+scale, DMA out.
"""

import sys

sys.path.insert(0, "/opt/trn_rl_repo")

from contextlib import ExitStack

import numpy as np

import concourse.bass as bass
import concourse.mybir as mybir
import concourse.tile as tile
from concourse import bacc
from concourse.bass import ds, ts
from concourse.bass_utils import run_bass_kernel_spmd

N_CORES = 8
N = 16384
D = 64
QR = N // N_CORES          # 2048 q rows per core
N_QTILE = QR // 128        # 16 q tiles per core
N_PHASE = 8                # groups of 2 q-tiles (256 q cols)
N_KV_BLK = N // 128        # 128 kv blocks
N_UNIT = 32                # units per phase (4 blocks each)
N_ACH = 16                 # 1024-key A-chunks per q tile
F32 = mybir.dt.float32
F8 = mybir.dt.float8e4
R32 = mybir.dt.float32r
DR = mybir.MatmulPerfMode.DoubleRow
AX = mybir.AxisListType.X
MAX = mybir.AluOpType.max

_CACHED = {}


def build_kernel():
    nc = bacc.Bacc("TRN2", target_bir_lowering=False, debug=False,
                   num_devices=N_CORES)

    x_d = nc.dram_tensor("x", [N, D], F32, kind="ExternalInput")
    xq_d = nc.dram_tensor("xq", [QR, D], F32, kind="ExternalInput")
    gm_d = nc.dram_tensor("gmat", [D + 1, D], F32, kind="ExternalInput")
    wv_d = nc.dram_tensor("wvb", [D + 1, D], F32, kind="ExternalInput")
    id_d = nc.dram_tensor("ident", [128, 128], F32, kind="ExternalInput")
    nid_d = nc.dram_tensor("nident", [128, 128], F32, kind="ExternalInput")
    oc_d = nc.dram_tensor("onescol", [128, 128], F32, kind="ExternalInput")
    y_d = nc.dram_tensor("y", [QR, D], F32, kind="ExternalOutput")

    def rb(ap):
        return ap.bitcast(R32)

    with tile.TileContext(nc) as tc, ExitStack() as ctx:
        sb = ctx.enter_context(tc.tile_pool(name="sb", bufs=1))
        expp = ctx.enter_context(tc.tile_pool(name="expp", bufs=4))
        smp = ctx.enter_context(tc.tile_pool(name="smp", bufs=4))
        scrp = ctx.enter_context(tc.tile_pool(name="scr", bufs=2))
        psB_p = ctx.enter_context(tc.tile_pool(name="psB", bufs=1, space="PSUM"))
        psA_p = ctx.enter_context(tc.tile_pool(name="psA", bufs=1, space="PSUM"))
        psPV_p = ctx.enter_context(tc.tile_pool(name="psPV", bufs=1, space="PSUM"))
        psM_p = ctx.enter_context(tc.tile_pool(name="psM", bufs=1, space="PSUM"))

        # ---- persistent SBUF ----
        xT = sb.tile([D + 1, N], F32)          # x^T, row 64 = ones
        x8 = sb.tile([32, 2, N], F8)           # DR d-split shadow of xT
        xV = sb.tile([128, N_KV_BLK, D + 1], F32)  # natural x, col 64 = ones
        xqn = sb.tile([128, N_QTILE, D + 1], F32)  # natural xq + ones col
        xqT = sb.tile([D + 1, QR], F32)        # xq^T, row 64 = ones
        gB = sb.tile([D + 1, QR], F32)         # g, row 64 = -m̂
        g8 = sb.tile([32, 2, QR], F8)
        gmat = sb.tile([D + 1, D], F32)
        wvb = sb.tile([D + 1, D], F32)
        ident = sb.tile([128, 128], F32)
        nident = sb.tile([128, 128], F32)
        onesb = sb.tile([128, 128], F32)
        mms = sb.tile([128, N_QTILE, 32], F32)
        # OT rows 0:64: out1 = sum p x, overwritten in-place by the
        # [Wv|bv]-stage at each phase tail; row 64 = denominator (sum p).
        OT = sb.tile([D + 1, QR], F32)
        y_sb = sb.tile([128, N_QTILE, D], F32)

        # ---- PSUM ----
        # The tile framework tracks PSUM hazards at tile granularity (and
        # marks PSUM-reading ACT/DVE ops as writers), so double-buffering
        # must use SEPARATE tiles, not halves of one tile.
        psBs = [psB_p.tile([128, 1024], F32, tag="psba", name="psba"),
                psB_p.tile([128, 1024], F32, tag="psbb", name="psbb")]
        # three-tile psA ring: reuse distance 3 keeps the single-tile
        # WAR chains from blocking the PE queue behind the DVE backlog.
        psAr = [psA_p.tile([128, 512], F32, tag=f"psa{i}", name=f"psa{i}")
                for i in range(3)]
        # psPV bank also hosts pneg (partition 127) - disjoint regions of
        # one tile; tile-granular hazards only serialize phase tails.
        psPV = psPV_p.tile([128, 512], F32)

        # ---- DMAs (DMA_ENGINES serializes; order by first use) ----
        nc.gpsimd.dma_start(ident[:], id_d[:])
        nc.gpsimd.dma_start(gmat[:].bitcast(R32), gm_d[:].bitcast(R32))
        nc.gpsimd.dma_start(wvb[:], wv_d[:])
        nc.gpsimd.dma_start(nident[:], nid_d[:])
        nc.gpsimd.dma_start(onesb[:], oc_d[:])
        nc.sync.dma_start(
            xqn[:, :, 0:D], xq_d[:].rearrange("(j p) d -> p j d", p=128))
        nc.gpsimd.memset(xqn[:, :, D : D + 1], 1.0)
        for c in range(8):
            nc.sync.dma_start(
                xV[:, ds(c * 16, 16), 0:D].bitcast(R32),
                x_d[ds(c * 2048, 2048), :].bitcast(R32).rearrange(
                    "(j p) d -> p j d", p=128))
        # r32 memset fails the ISA check; copy a DMA-loaded ones tile
        nc.vector.tensor_copy(rb(xV[:, :, D : D + 1]), onesb[:, :].rearrange("p (j o) -> p j o", o=1))

        # ---- xq transposes (borrow psB halves) + g-proj + g8 ----
        for o in range(2):
            tgt = psBs[o % 2]
            for j in range(8):
                nc.tensor.transpose(tgt[0 : D + 1, ts(j, 128)],
                                    xqn[:, o * 8 + j, :], ident[:])
            nc.vector.tensor_copy(rb(xqT[:, ts(o, 1024)]), tgt[0 : D + 1, :])
        for s in range(8):
            pm = psAr[s % 2][0:D, 0:256]
            nc.tensor.matmul(pm, rb(gmat[:]), rb(xqT[:, ts(s, 256)]),
                             start=True, stop=True)
            nc.vector.tensor_copy(rb(gB[0:D, ts(s, 256)]), pm)
        nc.scalar.copy(g8[:, 0, :], gB[0:32, :])
        nc.scalar.copy(g8[:, 1, :], gB[32:64, :])

        # ---- pass-A plumbing (512-key chunks on a 3-tile ring) ----
        a_ring = [0]

        def emit_A(t, c):
            r = psAr[a_ring[0] % 3]
            a_ring[0] += 1
            nc.tensor.matmul(r[:], g8[:, :, ts(t, 128)],
                             x8[:, :, ds(c * 512, 512)],
                             start=True, stop=True, perf_mode=DR)
            nc.vector.reduce_max(mms[:, t, c : c + 1], r[:], axis=AX)

        def emit_mfin(t):
            mt = smp.tile([128, 1], F32, tag="mt")
            nc.vector.reduce_max(mt[:], mms[:, t, :], axis=AX)
            # pneg = mt^T @ (-I) = -m̂ row; plain DVE copy into gB row 64
            # keeps the hot ACT queue out of the phase-boundary chain.
            pneg = psPV[64:65, 256:384]
            nc.tensor.matmul(pneg, mt[:], nident[:], start=True, stop=True)
            nc.vector.tensor_copy(rb(gB[D : D + 1, ts(t, 128)]), pneg)

        # ---- setup: kv chunks -> xT, x8, pass A for group 0 ----
        for c in range(8):
            for oo in range(2):
                o = 2 * c + oo
                tgt = psBs[o % 2]
                for j in range(8):
                    nc.tensor.transpose(tgt[0 : D + 1, ts(j, 128)],
                                        xV[:, o * 8 + j, :], ident[:])
                nc.scalar.copy(rb(xT[:, ts(o, 1024)]), tgt[0 : D + 1, :])
                nc.scalar.copy(x8[:, 0, ts(o, 1024)], tgt[0:32, :])
                # Pool cannot read PSUM: slot-1 shadow reads xT after its copy
                nc.gpsimd.tensor_copy(x8[:, 1, ts(o, 1024)],
                                      xT[32:64, ts(o, 1024)])
                # group-0 A work: oct o = keys [1024*o, 1024*o+1024):
                # chunks 2o, 2o+1 for tiles 0 and 1, ring-interleaved.
                for k in (2 * o, 2 * o + 1):
                    emit_A(0, k)
                    emit_A(1, k)
        emit_mfin(0)
        emit_mfin(1)

        # ---- phases ----
        for h in range(N_PHASE):
            qsl = ds(h * 256, 256)
            atiles = (2 * (h + 1), 2 * (h + 1) + 1) if h < N_PHASE - 1 else None
            pv_q = []   # batches of (blk, ex_ap); popped two units later
            # A-work queue: per tile 8 big + 16 sm ops; interleave
            # big/sm so the single-buffered tiles get WAR slack. Front-load
            # slightly so mfin chains clear before the next phase starts.
            a_q = []
            if atiles is not None:
                a_q = [(t, c) for t in atiles for c in range(32)]
            a_done = {}

            def emit_exp(u):
                ex = expp.tile([128, 1024], F32, tag="ex")
                nc.scalar.activation(rb(ex[:]), psBs[u % 2][:],
                                     mybir.ActivationFunctionType.Exp)
                pv_q.append([(4 * u + j, ex[:, ds(j * 256, 256)])
                             for j in range(4)])

            def emit_pv():
                for blk, ex_ap in pv_q.pop(0):
                    nc.tensor.matmul(psPV[0 : D + 1, 0:256], rb(xV[:, blk, :]),
                                     rb(ex_ap), start=(blk == 0),
                                     stop=(blk == N_KV_BLK - 1),
                                     skip_group_check=True)

            for u in range(N_UNIT):
                # Bs first: they feed this unit's exp with no other deps.
                for j in range(4):
                    blk = 4 * u + j
                    nc.tensor.matmul(psBs[u % 2][:, ds(j * 256, 256)],
                                     rb(xT[:, ts(blk, 128)]), rb(gB[:, qsl]),
                                     start=True, stop=True)
                # PVs of unit u-2: their exp finished during unit u-1.
                if len(pv_q) >= 2:
                    emit_pv()
                n_a = 0 if u == 0 else (3 if u < 6 else 2)
                for _ in range(min(n_a, len(a_q))):
                    t, k = a_q.pop(0)
                    emit_A(t, k)
                    a_done[t] = a_done.get(t, 0) + 1
                    if a_done[t] == 32:
                        emit_mfin(t)
                emit_exp(u)
            # tail: mfin chains are already emitted; flush PVs, save out1
            while pv_q:
                emit_pv()
            nc.scalar.copy(rb(OT[:, qsl]), psPV[0 : D + 1, 0:256])
            # [Wv|bv]-stage for this group, written back into OT rows 0:64;
            # then the denominator row becomes its reciprocal in place, so
            # the final transpose carries 1/denom in column 64. Exact fp32:
            # 8 matmuls of 256 cols, precision is worth the 4x row cost.
            pm = psAr[2][0:D, 256:512]
            nc.tensor.matmul(pm, wvb[:], OT[:, qsl],
                             start=True, stop=True)
            nc.scalar.copy(rb(OT[0:D, qsl]), pm)
            # reciprocal can't emit fp32r; bounce through scratch + rb copy
            rrow = scrp.tile([128, 256], F32, tag="scr")
            nc.vector.reciprocal(rrow[0:1, :], OT[D : D + 1, qsl])
            nc.vector.tensor_copy(rb(OT[D : D + 1, qsl]), rrow[0:1, :])

        # ---- final: transpose OT (col 64 = 1/denom), normalize, store ----
        for t in range(N_QTILE):
            pO = psBs[t % 2][:, 0 : D + 1]
            nc.tensor.transpose(pO, OT[:, ts(t, 128)],
                                ident[0 : D + 1, 0 : D + 1])
            nc.vector.tensor_scalar_mul(y_sb[:, t, :], pO[:, 0:D],
                                        pO[:, D : D + 1])
            if t % 4 == 3:
                nc.sync.dma_start(
                    y_d[ds((t - 3) * 128, 512), :].rearrange(
                        "(t p) d -> p t d", p=128),
                    y_sb[:, ds(t - 3, 4), :])

    nc.compile()
    return nc


def _prep_inputs(x, params, Wq, bq, Wk, bk, Wv, bv):
    f8s = np.float64
    x = np.ascontiguousarray(x, dtype=np.float32)
    params = np.asarray(params, f8s)
    rot = params[:, :D]
    ent = params[:, D : 2 * D]
    scale = 1.0 / np.sqrt(D)
    Wqp = (np.asarray(Wq, f8s) @ rot) * scale
    Wkp = np.asarray(Wk, f8s) @ ent
    bqs = np.asarray(bq, f8s) * scale
    G = Wkp.T @ Wqp
    w = Wkp.T @ bqs
    gmat = np.ascontiguousarray(
        np.vstack([G.T, w[None, :]]).astype(np.float32))
    wvb = np.ascontiguousarray(
        np.vstack([np.asarray(Wv, np.float32).T,
                   np.asarray(bv, np.float32)[None]]))
    ident = np.eye(128, dtype=np.float32)
    return x, gmat, wvb, ident


def kernel(x, params, Wq, bq, Wk, bk, Wv, bv, _trace=False):
    x, gmat, wvb, ident = _prep_inputs(x, params, Wq, bq, Wk, bk, Wv, bv)
    if "nc" not in _CACHED:
        _CACHED["nc"] = build_kernel()
    nc = _CACHED["nc"]
    in_maps = []
    for c in range(N_CORES):
        in_maps.append({
            "x": x,
            "xq": np.ascontiguousarray(x[c * QR : (c + 1) * QR]),
            "gmat": gmat, "wvb": wvb, "ident": ident, "nident": -ident,
            "onescol": np.ones([128, 128], np.float32),
        })
    res = run_bass_kernel_spmd(nc, in_maps, core_ids=list(range(N_CORES)),
                               trace=_trace)
    out = np.concatenate([res.results[c]["y"] for c in range(N_CORES)], axis=0)
    global _CACHED_RES
    _CACHED_RES = res
    return out


# revision 48
# speedup vs baseline: 1.1272x; 1.0152x over previous
"""Classical self-attention on 8 trn2 NeuronCores — v3.

N=16384 tokens, d=64, fp32. Sequence-parallel over Q: core c handles rows
[c*2048, (c+1)*2048). Per-core differentiation via the xq input slice.

Algebra (host-folded):
  s_jq = k_j . q_q = x_j^T (G x_q + w) + const_q, G/w precomputed on host;
  the per-q const is absorbed by the softmax shift, so it is never computed.
  sum_j p_j v_j = [Wv|bv] @ (sum_j p_j [x_j;1])  -> V never materialized;
  the ones column of xh doubles as the denominator row.

Structure per core:
  setup: DMA x (8 chunks) + xq; PE-transpose x -> xT[65,16384] (ones row);
    fp8 shadows x8[32,2,N] (DoubleRow d-split: d = slot*32 + partition);
    g-proj gB[65,2048] = [G^T;w^T] @ xqT, fp8 shadow g8; pass A for group 0.
  pass A (row max): fp8 DR matmuls psA[128,1024] per (tile, 1024-key chunk);
    DVE tensor_tensor_reduce (max,max) -> mms[:,t,c]; mfin: reduce -> mt,
    PE transpose -> gB row 64 = -m̂.
  8 phases (groups of 256 q = 2 tiles): per unit (4 kv blocks):
    PV(u-2) x4, B x4 (fp32r, psB[:,u%2,:]), exp(u-1) on ACT, TTR(u-1),
    A-mm(u) for group h+1 last (spacing hides the single-buffered psA WAR).
  tail: psPV[65,256] -> OT; after phase 7: OTw = [Wv|bv]^T @ OT (+denom row),
    PE transpose, DVE reciprocal+scale, DMA out.
"""

import sys

sys.path.insert(0, "/opt/trn_rl_repo")

from contextlib import ExitStack

import numpy as np

import concourse.bass as bass
import concourse.mybir as mybir
import concourse.tile as tile
from concourse import bacc
from concourse.bass import ds, ts
from concourse.bass_utils import run_bass_kernel_spmd

N_CORES = 8
N = 16384
D = 64
QR = N // N_CORES          # 2048 q rows per core
N_QTILE = QR // 128        # 16 q tiles per core
N_PHASE = 8                # groups of 2 q-tiles (256 q cols)
N_KV_BLK = N // 128        # 128 kv blocks
N_UNIT = 32                # units per phase (4 blocks each)
N_ACH = 16                 # 1024-key A-chunks per q tile
F32 = mybir.dt.float32
F8 = mybir.dt.float8e4
R32 = mybir.dt.float32r
DR = mybir.MatmulPerfMode.DoubleRow
AX = mybir.AxisListType.X
MAX = mybir.AluOpType.max

_CACHED = {}


def build_kernel():
    nc = bacc.Bacc("TRN2", target_bir_lowering=False, debug=False,
                   num_devices=N_CORES)

    x_d = nc.dram_tensor("x", [N, D], F32, kind="ExternalInput")
    xq_d = nc.dram_tensor("xq", [QR, D], F32, kind="ExternalInput")
    gm_d = nc.dram_tensor("gmat", [D + 1, D], F32, kind="ExternalInput")
    wv_d = nc.dram_tensor("wvb", [D + 1, D], F32, kind="ExternalInput")
    id_d = nc.dram_tensor("ident", [128, 128], F32, kind="ExternalInput")
    nid_d = nc.dram_tensor("nident", [128, 128], F32, kind="ExternalInput")
    oc_d = nc.dram_tensor("onescol", [128, 128], F32, kind="ExternalInput")
    y_d = nc.dram_tensor("y", [QR, D], F32, kind="ExternalOutput")

    def rb(ap):
        return ap.bitcast(R32)

    with tile.TileContext(nc) as tc, ExitStack() as ctx:
        sb = ctx.enter_context(tc.tile_pool(name="sb", bufs=1))
        expp = ctx.enter_context(tc.tile_pool(name="expp", bufs=4))
        smp = ctx.enter_context(tc.tile_pool(name="smp", bufs=4))
        scrp = ctx.enter_context(tc.tile_pool(name="scr", bufs=2))
        psB_p = ctx.enter_context(tc.tile_pool(name="psB", bufs=1, space="PSUM"))
        psA_p = ctx.enter_context(tc.tile_pool(name="psA", bufs=1, space="PSUM"))
        psPV_p = ctx.enter_context(tc.tile_pool(name="psPV", bufs=1, space="PSUM"))
        psM_p = ctx.enter_context(tc.tile_pool(name="psM", bufs=1, space="PSUM"))

        # ---- persistent SBUF ----
        xT = sb.tile([D + 1, N], F32)          # x^T, row 64 = ones
        x8 = sb.tile([32, 2, N], F8)           # DR d-split shadow of xT
        xV = sb.tile([128, N_KV_BLK, D + 1], F32)  # natural x, col 64 = ones
        xqn = sb.tile([128, N_QTILE, D + 1], F32)  # natural xq + ones col
        xqT = sb.tile([D + 1, QR], F32)        # xq^T, row 64 = ones
        gB = sb.tile([D + 1, QR], F32)         # g, row 64 = -m̂
        g8 = sb.tile([32, 2, QR], F8)
        gmat = sb.tile([D + 1, D], F32)
        wvb = sb.tile([D + 1, D], F32)
        ident = sb.tile([128, 128], F32)
        nident = sb.tile([128, 128], F32)
        onesb = sb.tile([128, 128], F32)
        mms = sb.tile([128, N_QTILE, 32], F32)
        msum = sb.tile([128, N_QTILE, 4], F32)   # ACT-lse partial sums
        # OT rows 0:64: out1 = sum p x, overwritten in-place by the
        # [Wv|bv]-stage at each phase tail; row 64 = denominator (sum p).
        OT = sb.tile([D + 1, QR], F32)
        y_sb = sb.tile([128, N_QTILE, D], F32)

        # ---- PSUM ----
        # The tile framework tracks PSUM hazards at tile granularity (and
        # marks PSUM-reading ACT/DVE ops as writers), so double-buffering
        # must use SEPARATE tiles, not halves of one tile.
        psBs = [psB_p.tile([128, 1024], F32, tag="psba", name="psba"),
                psB_p.tile([128, 1024], F32, tag="psbb", name="psbb")]
        # three-tile psA ring: reuse distance 3 keeps the single-tile
        # WAR chains from blocking the PE queue behind the DVE backlog.
        psAr = [psA_p.tile([128, 512], F32, tag=f"psa{i}", name=f"psa{i}")
                for i in range(3)]
        # psPV bank also hosts pneg (partition 127) - disjoint regions of
        # one tile; tile-granular hazards only serialize phase tails.
        psPV = psPV_p.tile([128, 512], F32)

        # ---- DMAs (DMA_ENGINES serializes; order by first use) ----
        nc.gpsimd.dma_start(ident[:], id_d[:])
        nc.gpsimd.dma_start(gmat[:].bitcast(R32), gm_d[:].bitcast(R32))
        nc.gpsimd.dma_start(wvb[:], wv_d[:])
        nc.gpsimd.dma_start(nident[:], nid_d[:])
        nc.gpsimd.dma_start(onesb[:], oc_d[:])
        nc.sync.dma_start(
            xqn[:, :, 0:D], xq_d[:].rearrange("(j p) d -> p j d", p=128))
        nc.gpsimd.memset(xqn[:, :, D : D + 1], 1.0)
        for c in range(8):
            nc.sync.dma_start(
                xV[:, ds(c * 16, 16), 0:D].bitcast(R32),
                x_d[ds(c * 2048, 2048), :].bitcast(R32).rearrange(
                    "(j p) d -> p j d", p=128))
        nc.gpsimd.memset(mms[:], -3.0e38)
        nc.gpsimd.memset(msum[:], 1.0e-30)
        # r32 memset fails the ISA check; copy a DMA-loaded ones tile
        nc.vector.tensor_copy(rb(xV[:, :, D : D + 1]), onesb[:, :].rearrange("p (j o) -> p j o", o=1))

        # ---- xq transposes (borrow psB halves) + g-proj + g8 ----
        for o in range(2):
            tgt = psBs[o % 2]
            for j in range(8):
                nc.tensor.transpose(tgt[0 : D + 1, ts(j, 128)],
                                    xqn[:, o * 8 + j, :], ident[:])
            nc.vector.tensor_copy(rb(xqT[:, ts(o, 1024)]), tgt[0 : D + 1, :])
        for s in range(8):
            pm = psAr[s % 2][0:D, 0:256]
            nc.tensor.matmul(pm, rb(gmat[:]), rb(xqT[:, ts(s, 256)]),
                             start=True, stop=True)
            nc.vector.tensor_copy(rb(gB[0:D, ts(s, 256)]), pm)
        nc.scalar.copy(g8[:, 0, :], gB[0:32, :])
        nc.scalar.copy(g8[:, 1, :], gB[32:64, :])

        # ---- pass-A plumbing (512-key chunks on a 3-tile ring) ----
        a_ring = [0]

        LSE_CHUNKS = ()   # bisect: disable ACT-lse

        def emit_A(t, c, allow_lse=True):
            r = psAr[a_ring[0] % 3]
            a_ring[0] += 1
            nc.tensor.matmul(r[:], g8[:, :, ts(t, 128)],
                             x8[:, :, ds(c * 512, 512)],
                             start=True, stop=True, perf_mode=DR)
            if allow_lse and c in LSE_CHUNKS:
                # ACT: accum = sum exp(s/8); 8*log(sum) bounds the chunk max
                # within +8*ln(512). Keeps ~10% of the max scan off DVE.
                ls = scrp.tile([128, 512], F32, tag="lscr")
                nc.scalar.activation(ls[:], r[:],
                                     mybir.ActivationFunctionType.Exp,
                                     scale=0.125,
                                     accum_out=msum[:, t, LSE_CHUNKS.index(c)
                                                    : LSE_CHUNKS.index(c) + 1])
            else:
                nc.vector.reduce_max(mms[:, t, c : c + 1], r[:], axis=AX)

        def emit_mfin(t):
            mt = smp.tile([128, 1], F32, tag="mt")
            nc.vector.reduce_max(mt[:], mms[:, t, :], axis=AX)
            # fold in the lse cells: m2 = 8*ln(max sums) via the bitcast
            # log2 approximation (error well inside the bound slack)
            sm = smp.tile([128, 1], F32, tag="sm")
            nc.vector.reduce_max(sm[:], msum[:, t, :], axis=AX)
            smf = smp.tile([128, 1], F32, tag="smf")
            nc.vector.tensor_copy(smf[:], sm[:].bitcast(mybir.dt.int32))
            nc.vector.tensor_scalar(smf[:], smf[:], 8 * 0.6931472 / 2 ** 23,
                                    -126.9 * 8 * 0.6931472,
                                    op0=mybir.AluOpType.mult,
                                    op1=mybir.AluOpType.add)
            nc.vector.tensor_tensor(mt[:], mt[:], smf[:],
                                    op=mybir.AluOpType.max)
            # pneg = mt^T @ (-I) = -m̂ row; runs as a psA-ring op so no
            # live psum region is disturbed (a start=True matmul must not
            # share a bank with an in-flight accumulation group).
            r = psAr[a_ring[0] % 3]
            a_ring[0] += 1
            pneg = r[0:1, 0:128]
            nc.tensor.matmul(pneg, mt[:], nident[:], start=True, stop=True)
            nc.vector.tensor_copy(rb(gB[D : D + 1, ts(t, 128)]), pneg)

        # ---- setup: kv chunks -> xT, x8, pass A for group 0 ----
        for c in range(8):
            for oo in range(2):
                o = 2 * c + oo
                tgt = psBs[o % 2]
                for j in range(8):
                    nc.tensor.transpose(tgt[0 : D + 1, ts(j, 128)],
                                        xV[:, o * 8 + j, :], ident[:])
                nc.scalar.copy(rb(xT[:, ts(o, 1024)]), tgt[0 : D + 1, :])
                nc.scalar.copy(x8[:, 0, ts(o, 1024)], tgt[0:32, :])
                # Pool cannot read PSUM: slot-1 shadow reads xT after its copy
                nc.gpsimd.tensor_copy(x8[:, 1, ts(o, 1024)],
                                      xT[32:64, ts(o, 1024)])
                # group-0 A work: oct o = keys [1024*o, 1024*o+1024):
                # chunks 2o, 2o+1 for tiles 0 and 1, ring-interleaved.
                for k in (2 * o, 2 * o + 1):
                    emit_A(0, k, allow_lse=False)
                    emit_A(1, k, allow_lse=False)
        emit_mfin(0)
        emit_mfin(1)

        # ---- phases ----
        for h in range(N_PHASE):
            qsl = ds(h * 256, 256)
            atiles = (2 * (h + 1), 2 * (h + 1) + 1) if h < N_PHASE - 1 else None
            pv_q = []   # batches of (blk, ex_ap); popped two units later
            # A-work queue: per tile 8 big + 16 sm ops; interleave
            # big/sm so the single-buffered tiles get WAR slack. Front-load
            # slightly so mfin chains clear before the next phase starts.
            a_q = []
            if atiles is not None:
                a_q = [(t, c) for t in atiles for c in range(32)]
            a_done = {}

            def emit_exp(u):
                ex = expp.tile([128, 1024], F32, tag="ex")
                nc.scalar.activation(rb(ex[:]), psBs[u % 2][:],
                                     mybir.ActivationFunctionType.Exp)
                pv_q.append([(4 * u + j, ex[:, ds(j * 256, 256)])
                             for j in range(4)])

            def emit_pv():
                for blk, ex_ap in pv_q.pop(0):
                    nc.tensor.matmul(psPV[0 : D + 1, 0:256], rb(xV[:, blk, :]),
                                     rb(ex_ap), start=(blk == 0),
                                     stop=(blk == N_KV_BLK - 1),
                                     skip_group_check=True)

            for u in range(N_UNIT):
                # Bs first: they feed this unit's exp with no other deps.
                for j in range(4):
                    blk = 4 * u + j
                    nc.tensor.matmul(psBs[u % 2][:, ds(j * 256, 256)],
                                     rb(xT[:, ts(blk, 128)]), rb(gB[:, qsl]),
                                     start=True, stop=True)
                # PVs of unit u-2: their exp finished during unit u-1.
                if len(pv_q) >= 2:
                    emit_pv()
                n_a = 0 if u == 0 else (3 if u < 6 else 2)
                for _ in range(min(n_a, len(a_q))):
                    t, k = a_q.pop(0)
                    emit_A(t, k)
                    a_done[t] = a_done.get(t, 0) + 1
                    if a_done[t] == 32:
                        emit_mfin(t)
                emit_exp(u)
            # tail: mfin chains are already emitted; flush PVs, save out1
            while pv_q:
                emit_pv()
            nc.scalar.copy(rb(OT[:, qsl]), psPV[0 : D + 1, 0:256])
            # [Wv|bv]-stage for this group, written back into OT rows 0:64;
            # then the denominator row becomes its reciprocal in place, so
            # the final transpose carries 1/denom in column 64. Exact fp32:
            # 8 matmuls of 256 cols, precision is worth the 4x row cost.
            pm = psAr[2][0:D, 256:512]
            nc.tensor.matmul(pm, wvb[:], OT[:, qsl],
                             start=True, stop=True)
            nc.scalar.copy(rb(OT[0:D, qsl]), pm)
            # reciprocal can't emit fp32r; bounce through scratch + rb copy
            rrow = scrp.tile([128, 256], F32, tag="scr")
            nc.vector.reciprocal(rrow[0:1, :], OT[D : D + 1, qsl])
            nc.vector.tensor_copy(rb(OT[D : D + 1, qsl]), rrow[0:1, :])

        # ---- final: transpose OT (col 64 = 1/denom), normalize, store ----
        for t in range(N_QTILE):
            pO = psBs[t % 2][:, 0 : D + 1]
            nc.tensor.transpose(pO, OT[:, ts(t, 128)],
                                ident[0 : D + 1, 0 : D + 1])
            nc.vector.tensor_scalar_mul(y_sb[:, t, :], pO[:, 0:D],
                                        pO[:, D : D + 1])
            if t % 4 == 3:
                nc.sync.dma_start(
                    y_d[ds((t - 3) * 128, 512), :].rearrange(
                        "(t p) d -> p t d", p=128),
                    y_sb[:, ds(t - 3, 4), :])

    nc.compile()
    return nc


def _prep_inputs(x, params, Wq, bq, Wk, bk, Wv, bv):
    f8s = np.float64
    x = np.ascontiguousarray(x, dtype=np.float32)
    params = np.asarray(params, f8s)
    rot = params[:, :D]
    ent = params[:, D : 2 * D]
    scale = 1.0 / np.sqrt(D)
    Wqp = (np.asarray(Wq, f8s) @ rot) * scale
    Wkp = np.asarray(Wk, f8s) @ ent
    bqs = np.asarray(bq, f8s) * scale
    G = Wkp.T @ Wqp
    w = Wkp.T @ bqs
    gmat = np.ascontiguousarray(
        np.vstack([G.T, w[None, :]]).astype(np.float32))
    wvb = np.ascontiguousarray(
        np.vstack([np.asarray(Wv, np.float32).T,
                   np.asarray(bv, np.float32)[None]]))
    ident = np.eye(128, dtype=np.float32)
    return x, gmat, wvb, ident


def kernel(x, params, Wq, bq, Wk, bk, Wv, bv, _trace=False):
    x, gmat, wvb, ident = _prep_inputs(x, params, Wq, bq, Wk, bk, Wv, bv)
    if "nc" not in _CACHED:
        _CACHED["nc"] = build_kernel()
    nc = _CACHED["nc"]
    in_maps = []
    for c in range(N_CORES):
        in_maps.append({
            "x": x,
            "xq": np.ascontiguousarray(x[c * QR : (c + 1) * QR]),
            "gmat": gmat, "wvb": wvb, "ident": ident, "nident": -ident,
            "onescol": np.ones([128, 128], np.float32),
        })
    res = run_bass_kernel_spmd(nc, in_maps, core_ids=list(range(N_CORES)),
                               trace=_trace)
    out = np.concatenate([res.results[c]["y"] for c in range(N_CORES)], axis=0)
    global _CACHED_RES
    _CACHED_RES = res
    return out


# revision 49
# speedup vs baseline: 1.1546x; 1.0243x over previous
"""Classical self-attention on 8 trn2 NeuronCores — v3.

N=16384 tokens, d=64, fp32. Sequence-parallel over Q: core c handles rows
[c*2048, (c+1)*2048). Per-core differentiation via the xq input slice.

Algebra (host-folded):
  s_jq = k_j . q_q = x_j^T (G x_q + w) + const_q, G/w precomputed on host;
  the per-q const is absorbed by the softmax shift, so it is never computed.
  sum_j p_j v_j = [Wv|bv] @ (sum_j p_j [x_j;1])  -> V never materialized;
  the ones column of xh doubles as the denominator row.

Structure per core:
  setup: DMA x (8 chunks) + xq; PE-transpose x -> xT[65,16384] (ones row);
    fp8 shadows x8[32,2,N] (DoubleRow d-split: d = slot*32 + partition);
    g-proj gB[65,2048] = [G^T;w^T] @ xqT, fp8 shadow g8; pass A for group 0.
  pass A (row max): fp8 DR matmuls psA[128,1024] per (tile, 1024-key chunk);
    DVE tensor_tensor_reduce (max,max) -> mms[:,t,c]; mfin: reduce -> mt,
    PE transpose -> gB row 64 = -m̂.
  8 phases (groups of 256 q = 2 tiles): per unit (4 kv blocks):
    PV(u-2) x4, B x4 (fp32r, psB[:,u%2,:]), exp(u-1) on ACT, TTR(u-1),
    A-mm(u) for group h+1 last (spacing hides the single-buffered psA WAR).
  tail: psPV[65,256] -> OT; after phase 7: OTw = [Wv|bv]^T @ OT (+denom row),
    PE transpose, DVE reciprocal+scale, DMA out.
"""

import sys

sys.path.insert(0, "/opt/trn_rl_repo")

from contextlib import ExitStack

import numpy as np

import concourse.bass as bass
import concourse.mybir as mybir
import concourse.tile as tile
from concourse import bacc
from concourse.bass import ds, ts
from concourse.bass_utils import run_bass_kernel_spmd

N_CORES = 8
N = 16384
D = 64
QR = N // N_CORES          # 2048 q rows per core
N_QTILE = QR // 128        # 16 q tiles per core
N_PHASE = 8                # groups of 2 q-tiles (256 q cols)
N_KV_BLK = N // 128        # 128 kv blocks
N_UNIT = 32                # units per phase (4 blocks each)
N_ACH = 16                 # 1024-key A-chunks per q tile
F32 = mybir.dt.float32
F8 = mybir.dt.float8e4
R32 = mybir.dt.float32r
DR = mybir.MatmulPerfMode.DoubleRow
AX = mybir.AxisListType.X
MAX = mybir.AluOpType.max

_CACHED = {}


def build_kernel():
    nc = bacc.Bacc("TRN2", target_bir_lowering=False, debug=False,
                   num_devices=N_CORES)

    x_d = nc.dram_tensor("x", [N, D], F32, kind="ExternalInput")
    xq_d = nc.dram_tensor("xq", [QR, D], F32, kind="ExternalInput")
    gm_d = nc.dram_tensor("gmat", [D + 1, D], F32, kind="ExternalInput")
    wv_d = nc.dram_tensor("wvb", [D + 1, D], F32, kind="ExternalInput")
    id_d = nc.dram_tensor("ident", [128, 128], F32, kind="ExternalInput")
    nid_d = nc.dram_tensor("nident", [128, 128], F32, kind="ExternalInput")
    oc_d = nc.dram_tensor("onescol", [128, 128], F32, kind="ExternalInput")
    y_d = nc.dram_tensor("y", [QR, D], F32, kind="ExternalOutput")

    def rb(ap):
        return ap.bitcast(R32)

    with tile.TileContext(nc) as tc, ExitStack() as ctx:
        sb = ctx.enter_context(tc.tile_pool(name="sb", bufs=1))
        expp = ctx.enter_context(tc.tile_pool(name="expp", bufs=4))
        smp = ctx.enter_context(tc.tile_pool(name="smp", bufs=4))
        scrp = ctx.enter_context(tc.tile_pool(name="scr", bufs=2))
        psB_p = ctx.enter_context(tc.tile_pool(name="psB", bufs=1, space="PSUM"))
        psA_p = ctx.enter_context(tc.tile_pool(name="psA", bufs=1, space="PSUM"))
        psPV_p = ctx.enter_context(tc.tile_pool(name="psPV", bufs=1, space="PSUM"))
        psM_p = ctx.enter_context(tc.tile_pool(name="psM", bufs=1, space="PSUM"))

        # ---- persistent SBUF ----
        xT = sb.tile([D + 1, N], F32)          # x^T, row 64 = ones
        x8 = sb.tile([32, 2, N], F8)           # DR d-split shadow of xT
        xV = sb.tile([128, N_KV_BLK, D + 1], F32)  # natural x, col 64 = ones
        xqn = sb.tile([128, N_QTILE, D + 1], F32)  # natural xq + ones col
        xqT = sb.tile([D + 1, QR], F32)        # xq^T, row 64 = ones
        gB = sb.tile([D + 1, QR], F32)         # g, row 64 = -m̂
        g8 = sb.tile([32, 2, QR], F8)
        gmat = sb.tile([D + 1, D], F32)
        wvb = sb.tile([D + 1, D], F32)
        ident = sb.tile([128, 128], F32)
        nident = sb.tile([128, 128], F32)
        onesb = sb.tile([128, 128], F32)
        mms = sb.tile([128, N_QTILE, 32], F32)
        msum = sb.tile([128, N_QTILE, 4], F32)   # ACT-lse partial sums
        # OT rows 0:64: out1 = sum p x, overwritten in-place by the
        # [Wv|bv]-stage at each phase tail; row 64 = denominator (sum p).
        OT = sb.tile([D + 1, QR], F32)
        y_sb = sb.tile([128, N_QTILE, D], F32)

        # ---- PSUM ----
        # The tile framework tracks PSUM hazards at tile granularity (and
        # marks PSUM-reading ACT/DVE ops as writers), so double-buffering
        # must use SEPARATE tiles, not halves of one tile.
        psBs = [psB_p.tile([128, 1024], F32, tag="psba", name="psba"),
                psB_p.tile([128, 1024], F32, tag="psbb", name="psbb")]
        # three-tile psA ring: reuse distance 3 keeps the single-tile
        # WAR chains from blocking the PE queue behind the DVE backlog.
        psAr = [psA_p.tile([128, 512], F32, tag=f"psa{i}", name=f"psa{i}")
                for i in range(3)]
        # psPV bank also hosts pneg (partition 127) - disjoint regions of
        # one tile; tile-granular hazards only serialize phase tails.
        psPV = psPV_p.tile([128, 512], F32)

        # ---- DMAs (DMA_ENGINES serializes; order by first use) ----
        nc.gpsimd.dma_start(ident[:], id_d[:])
        nc.gpsimd.dma_start(gmat[:].bitcast(R32), gm_d[:].bitcast(R32))
        nc.gpsimd.dma_start(wvb[:], wv_d[:])
        nc.gpsimd.dma_start(nident[:], nid_d[:])
        nc.gpsimd.dma_start(onesb[:], oc_d[:])
        nc.sync.dma_start(
            xqn[:, :, 0:D], xq_d[:].rearrange("(j p) d -> p j d", p=128))
        nc.gpsimd.memset(xqn[:, :, D : D + 1], 1.0)
        for c in range(8):
            nc.sync.dma_start(
                xV[:, ds(c * 16, 16), 0:D].bitcast(R32),
                x_d[ds(c * 2048, 2048), :].bitcast(R32).rearrange(
                    "(j p) d -> p j d", p=128))
        nc.gpsimd.memset(mms[:], -3.0e38)
        nc.gpsimd.memset(msum[:], 1.0e-30)
        # r32 memset fails the ISA check; copy a DMA-loaded ones tile
        nc.vector.tensor_copy(rb(xV[:, :, D : D + 1]), onesb[:, :].rearrange("p (j o) -> p j o", o=1))

        # ---- xq transposes (borrow psB halves) + g-proj + g8 ----
        for o in range(2):
            tgt = psBs[o % 2]
            for j in range(8):
                nc.tensor.transpose(tgt[0 : D + 1, ts(j, 128)],
                                    xqn[:, o * 8 + j, :], ident[:])
            nc.vector.tensor_copy(rb(xqT[:, ts(o, 1024)]), tgt[0 : D + 1, :])
        for s in range(8):
            pm = psAr[s % 2][0:D, 0:256]
            nc.tensor.matmul(pm, rb(gmat[:]), rb(xqT[:, ts(s, 256)]),
                             start=True, stop=True)
            nc.vector.tensor_copy(rb(gB[0:D, ts(s, 256)]), pm)
        nc.scalar.copy(g8[:, 0, :], gB[0:32, :])
        nc.scalar.copy(g8[:, 1, :], gB[32:64, :])

        # ---- pass-A plumbing (512-key chunks on a 3-tile ring) ----
        a_ring = [0]

        LSE_CHUNKS = (1, 3, 5)   # handled by ACT as sum(exp(s/8))

        def emit_A(t, c, allow_lse=True):
            r = psAr[a_ring[0] % 3]
            a_ring[0] += 1
            nc.tensor.matmul(r[:], g8[:, :, ts(t, 128)],
                             x8[:, :, ds(c * 512, 512)],
                             start=True, stop=True, perf_mode=DR)
            if allow_lse and c in LSE_CHUNKS:
                # ACT: accum = sum exp(s/8); 8*log(sum) bounds the chunk max
                # within +8*ln(512). Keeps ~10% of the max scan off DVE.
                ls = scrp.tile([128, 512], F32, tag="lscr")
                nc.scalar.activation(ls[:], r[:],
                                     mybir.ActivationFunctionType.Exp,
                                     scale=0.125,
                                     accum_out=msum[:, t, LSE_CHUNKS.index(c)
                                                    : LSE_CHUNKS.index(c) + 1])
            else:
                nc.vector.reduce_max(mms[:, t, c : c + 1], r[:], axis=AX)

        def emit_mfin(t):
            mt = smp.tile([128, 1], F32, tag="mt")
            nc.vector.reduce_max(mt[:], mms[:, t, :], axis=AX)
            # fold in the lse cells: m2 = 8*ln(max sums) via the bitcast
            # log2 approximation (error well inside the bound slack)
            sm = smp.tile([128, 1], F32, tag="sm")
            nc.vector.reduce_max(sm[:], msum[:, t, :], axis=AX)
            smf = smp.tile([128, 1], F32, tag="smf")
            nc.vector.tensor_copy(smf[:], sm[:].bitcast(mybir.dt.int32))
            nc.vector.tensor_scalar(smf[:], smf[:], 8 * 0.6931472 / 2 ** 23,
                                    -126.9 * 8 * 0.6931472,
                                    op0=mybir.AluOpType.mult,
                                    op1=mybir.AluOpType.add)
            nc.vector.tensor_tensor(mt[:], mt[:], smf[:],
                                    op=mybir.AluOpType.max)
            # pneg = mt^T @ (-I) = -m̂ row; runs as a psA-ring op so no
            # live psum region is disturbed (a start=True matmul must not
            # share a bank with an in-flight accumulation group).
            r = psAr[a_ring[0] % 3]
            a_ring[0] += 1
            pneg = r[0:1, 0:128]
            nc.tensor.matmul(pneg, mt[:], nident[:], start=True, stop=True)
            nc.vector.tensor_copy(rb(gB[D : D + 1, ts(t, 128)]), pneg)

        # ---- setup: kv chunks -> xT, x8, pass A for group 0 ----
        for c in range(8):
            for oo in range(2):
                o = 2 * c + oo
                tgt = psBs[o % 2]
                for j in range(8):
                    nc.tensor.transpose(tgt[0 : D + 1, ts(j, 128)],
                                        xV[:, o * 8 + j, :], ident[:])
                nc.scalar.copy(rb(xT[:, ts(o, 1024)]), tgt[0 : D + 1, :])
                nc.scalar.copy(x8[:, 0, ts(o, 1024)], tgt[0:32, :])
                # Pool cannot read PSUM: slot-1 shadow reads xT after its copy
                nc.gpsimd.tensor_copy(x8[:, 1, ts(o, 1024)],
                                      xT[32:64, ts(o, 1024)])
                # group-0 A work: oct o = keys [1024*o, 1024*o+1024):
                # chunks 2o, 2o+1 for tiles 0 and 1, ring-interleaved.
                for k in (2 * o, 2 * o + 1):
                    emit_A(0, k, allow_lse=False)
                    emit_A(1, k, allow_lse=False)
        emit_mfin(0)
        emit_mfin(1)

        # ---- phases ----
        for h in range(N_PHASE):
            qsl = ds(h * 256, 256)
            atiles = (2 * (h + 1), 2 * (h + 1) + 1) if h < N_PHASE - 1 else None
            pv_q = []   # batches of (blk, ex_ap); popped two units later
            # A-work queue: per tile 8 big + 16 sm ops; interleave
            # big/sm so the single-buffered tiles get WAR slack. Front-load
            # slightly so mfin chains clear before the next phase starts.
            a_q = []
            if atiles is not None:
                a_q = [(t, c) for t in atiles for c in range(32)]
            a_done = {}

            def emit_exp(u):
                ex = expp.tile([128, 1024], F32, tag="ex")
                nc.scalar.activation(rb(ex[:]), psBs[u % 2][:],
                                     mybir.ActivationFunctionType.Exp)
                pv_q.append([(4 * u + j, ex[:, ds(j * 256, 256)])
                             for j in range(4)])

            def emit_pv():
                for blk, ex_ap in pv_q.pop(0):
                    nc.tensor.matmul(psPV[0 : D + 1, 0:256], rb(xV[:, blk, :]),
                                     rb(ex_ap), start=(blk == 0),
                                     stop=(blk == N_KV_BLK - 1),
                                     skip_group_check=True)

            for u in range(N_UNIT):
                # Bs first: they feed this unit's exp with no other deps.
                for j in range(4):
                    blk = 4 * u + j
                    nc.tensor.matmul(psBs[u % 2][:, ds(j * 256, 256)],
                                     rb(xT[:, ts(blk, 128)]), rb(gB[:, qsl]),
                                     start=True, stop=True)
                # PVs of unit u-2: their exp finished during unit u-1.
                if len(pv_q) >= 2:
                    emit_pv()
                n_a = 0 if u == 0 else (3 if u < 6 else 2)
                for _ in range(min(n_a, len(a_q))):
                    t, k = a_q.pop(0)
                    emit_A(t, k)
                    a_done[t] = a_done.get(t, 0) + 1
                    if a_done[t] == 32:
                        emit_mfin(t)
                emit_exp(u)
            # tail: mfin chains are already emitted; flush PVs, save out1
            while pv_q:
                emit_pv()
            nc.scalar.copy(rb(OT[:, qsl]), psPV[0 : D + 1, 0:256])
            # [Wv|bv]-stage for this group, written back into OT rows 0:64;
            # then the denominator row becomes its reciprocal in place, so
            # the final transpose carries 1/denom in column 64. Exact fp32:
            # 8 matmuls of 256 cols, precision is worth the 4x row cost.
            pm = psAr[2][0:D, 256:512]
            nc.tensor.matmul(pm, wvb[:], OT[:, qsl],
                             start=True, stop=True)
            nc.scalar.copy(rb(OT[0:D, qsl]), pm)
            # reciprocal can't emit fp32r; bounce through scratch + rb copy
            rrow = scrp.tile([128, 256], F32, tag="scr")
            nc.vector.reciprocal(rrow[0:1, :], OT[D : D + 1, qsl])
            nc.vector.tensor_copy(rb(OT[D : D + 1, qsl]), rrow[0:1, :])

        # ---- final: transpose OT (col 64 = 1/denom), normalize, store ----
        for t in range(N_QTILE):
            pO = psBs[t % 2][:, 0 : D + 1]
            nc.tensor.transpose(pO, OT[:, ts(t, 128)],
                                ident[0 : D + 1, 0 : D + 1])
            nc.vector.tensor_scalar_mul(y_sb[:, t, :], pO[:, 0:D],
                                        pO[:, D : D + 1])
            if t % 4 == 3:
                nc.sync.dma_start(
                    y_d[ds((t - 3) * 128, 512), :].rearrange(
                        "(t p) d -> p t d", p=128),
                    y_sb[:, ds(t - 3, 4), :])

    nc.compile()
    return nc


def _prep_inputs(x, params, Wq, bq, Wk, bk, Wv, bv):
    f8s = np.float64
    x = np.ascontiguousarray(x, dtype=np.float32)
    params = np.asarray(params, f8s)
    rot = params[:, :D]
    ent = params[:, D : 2 * D]
    scale = 1.0 / np.sqrt(D)
    Wqp = (np.asarray(Wq, f8s) @ rot) * scale
    Wkp = np.asarray(Wk, f8s) @ ent
    bqs = np.asarray(bq, f8s) * scale
    G = Wkp.T @ Wqp
    w = Wkp.T @ bqs
    gmat = np.ascontiguousarray(
        np.vstack([G.T, w[None, :]]).astype(np.float32))
    wvb = np.ascontiguousarray(
        np.vstack([np.asarray(Wv, np.float32).T,
                   np.asarray(bv, np.float32)[None]]))
    ident = np.eye(128, dtype=np.float32)
    return x, gmat, wvb, ident


def kernel(x, params, Wq, bq, Wk, bk, Wv, bv, _trace=False):
    x, gmat, wvb, ident = _prep_inputs(x, params, Wq, bq, Wk, bk, Wv, bv)
    if "nc" not in _CACHED:
        _CACHED["nc"] = build_kernel()
    nc = _CACHED["nc"]
    in_maps = []
    for c in range(N_CORES):
        in_maps.append({
            "x": x,
            "xq": np.ascontiguousarray(x[c * QR : (c + 1) * QR]),
            "gmat": gmat, "wvb": wvb, "ident": ident, "nident": -ident,
            "onescol": np.ones([128, 128], np.float32),
        })
    res = run_bass_kernel_spmd(nc, in_maps, core_ids=list(range(N_CORES)),
                               trace=_trace)
    out = np.concatenate([res.results[c]["y"] for c in range(N_CORES)], axis=0)
    global _CACHED_RES
    _CACHED_RES = res
    return out


# revision 55
# speedup vs baseline: 1.1656x; 1.0095x over previous
"""Classical self-attention on 8 trn2 NeuronCores — v3.

N=16384 tokens, d=64, fp32. Sequence-parallel over Q: core c handles rows
[c*2048, (c+1)*2048). Per-core differentiation via the xq input slice.

Algebra (host-folded):
  s_jq = k_j . q_q = x_j^T (G x_q + w) + const_q, G/w precomputed on host;
  the per-q const is absorbed by the softmax shift, so it is never computed.
  sum_j p_j v_j = [Wv|bv] @ (sum_j p_j [x_j;1])  -> V never materialized;
  the ones column of xh doubles as the denominator row.

Structure per core:
  setup: DMA x (8 chunks) + xq; PE-transpose x -> xT[65,16384] (ones row);
    fp8 shadows x8[32,2,N] (DoubleRow d-split: d = slot*32 + partition);
    g-proj gB[65,2048] = [G^T;w^T] @ xqT, fp8 shadow g8; pass A for group 0.
  pass A (row max): fp8 DR matmuls psA[128,1024] per (tile, 1024-key chunk);
    DVE tensor_tensor_reduce (max,max) -> mms[:,t,c]; mfin: reduce -> mt,
    PE transpose -> gB row 64 = -m̂.
  8 phases (groups of 256 q = 2 tiles): per unit (4 kv blocks):
    PV(u-2) x4, B x4 (fp32r, psB[:,u%2,:]), exp(u-1) on ACT, TTR(u-1),
    A-mm(u) for group h+1 last (spacing hides the single-buffered psA WAR).
  tail: psPV[65,256] -> OT; after phase 7: OTw = [Wv|bv]^T @ OT (+denom row),
    PE transpose, DVE reciprocal+scale, DMA out.
"""

import sys

sys.path.insert(0, "/opt/trn_rl_repo")

from contextlib import ExitStack

import numpy as np

import concourse.bass as bass
import concourse.mybir as mybir
import concourse.tile as tile
from concourse import bacc
from concourse.bass import ds, ts
from concourse.bass_utils import run_bass_kernel_spmd

N_CORES = 8
N = 16384
D = 64
QR = N // N_CORES          # 2048 q rows per core
N_QTILE = QR // 128        # 16 q tiles per core
N_PHASE = 8                # groups of 2 q-tiles (256 q cols)
N_KV_BLK = N // 128        # 128 kv blocks
N_UNIT = 32                # units per phase (4 blocks each)
N_ACH = 16                 # 1024-key A-chunks per q tile
F32 = mybir.dt.float32
F8 = mybir.dt.float8e4
R32 = mybir.dt.float32r
DR = mybir.MatmulPerfMode.DoubleRow
AX = mybir.AxisListType.X
MAX = mybir.AluOpType.max

_CACHED = {}


def build_kernel():
    nc = bacc.Bacc("TRN2", target_bir_lowering=False, debug=False,
                   num_devices=N_CORES)

    x_d = nc.dram_tensor("x", [N, D], F32, kind="ExternalInput")
    xq_d = nc.dram_tensor("xq", [D + 1, QR], F32, kind="ExternalInput")
    gm_d = nc.dram_tensor("gmat", [D + 1, D], F32, kind="ExternalInput")
    wv_d = nc.dram_tensor("wvb", [D + 1, D], F32, kind="ExternalInput")
    id_d = nc.dram_tensor("ident", [128, 128], F32, kind="ExternalInput")
    nid_d = nc.dram_tensor("nident", [128, 128], F32, kind="ExternalInput")
    oc_d = nc.dram_tensor("onescol", [128, 128], F32, kind="ExternalInput")
    y_d = nc.dram_tensor("y", [QR, D], F32, kind="ExternalOutput")

    def rb(ap):
        return ap.bitcast(R32)

    with tile.TileContext(nc) as tc, ExitStack() as ctx:
        sb = ctx.enter_context(tc.tile_pool(name="sb", bufs=1))
        expp = ctx.enter_context(tc.tile_pool(name="expp", bufs=4))
        smp = ctx.enter_context(tc.tile_pool(name="smp", bufs=4))
        scrp = ctx.enter_context(tc.tile_pool(name="scr", bufs=2))
        psB_p = ctx.enter_context(tc.tile_pool(name="psB", bufs=1, space="PSUM"))
        psA_p = ctx.enter_context(tc.tile_pool(name="psA", bufs=1, space="PSUM"))
        psPV_p = ctx.enter_context(tc.tile_pool(name="psPV", bufs=1, space="PSUM"))
        psM_p = ctx.enter_context(tc.tile_pool(name="psM", bufs=1, space="PSUM"))

        # ---- persistent SBUF ----
        xT = sb.tile([D + 1, N], F32)          # x^T, row 64 = ones
        x8 = sb.tile([32, 2, N], F8)           # DR d-split shadow of xT
        xV = sb.tile([128, N_KV_BLK, D + 1], F32)  # natural x, col 64 = ones
        xqT = sb.tile([D + 1, QR], F32)        # xq^T, row 64 = ones
        gB = sb.tile([D + 1, QR], F32)         # g, row 64 = -m̂
        g8 = sb.tile([32, 2, QR], F8)
        gmat = sb.tile([D + 1, D], F32)
        wvb = sb.tile([D + 1, D], F32)
        ident = sb.tile([128, 128], F32)
        nident = sb.tile([128, 128], F32)
        onesb = sb.tile([128, 128], F32)
        nba = sb.tile([128, 1], F32)     # -B/A bias for phase-7 ACT exps
        mms = sb.tile([128, N_QTILE, 32], F32)
        msum = sb.tile([128, N_QTILE, 4], F32)   # ACT-lse partial sums
        # OT rows 0:64: out1 = sum p x, overwritten in-place by the
        # [Wv|bv]-stage at each phase tail; row 64 = denominator (sum p).
        OT = sb.tile([D + 1, QR], F32)
        y_sb = sb.tile([128, N_QTILE, D], F32)

        # ---- PSUM ----
        # The tile framework tracks PSUM hazards at tile granularity (and
        # marks PSUM-reading ACT/DVE ops as writers), so double-buffering
        # must use SEPARATE tiles, not halves of one tile.
        psBs = [psB_p.tile([128, 1024], F32, tag="psba", name="psba"),
                psB_p.tile([128, 1024], F32, tag="psbb", name="psbb")]
        # three-tile psA ring: reuse distance 3 keeps the single-tile
        # WAR chains from blocking the PE queue behind the DVE backlog.
        psAr = [psA_p.tile([128, 512], F32, tag=f"psa{i}", name=f"psa{i}")
                for i in range(3)]
        # psPV bank also hosts pneg (partition 127) - disjoint regions of
        # one tile; tile-granular hazards only serialize phase tails.
        psPV = psPV_p.tile([128, 512], F32)

        # ---- DMAs (DMA_ENGINES serializes; order by first use) ----
        nc.gpsimd.dma_start(ident[:], id_d[:])
        nc.gpsimd.dma_start(gmat[:].bitcast(R32), gm_d[:].bitcast(R32))
        nc.gpsimd.dma_start(wvb[:], wv_d[:])
        nc.gpsimd.dma_start(nident[:], nid_d[:])
        nc.gpsimd.dma_start(onesb[:], oc_d[:])
        nc.sync.dma_start(xqT[:].bitcast(R32), xq_d[:].bitcast(R32))
        for c in range(8):
            nc.sync.dma_start(
                xV[:, ds(c * 16, 16), 0:D].bitcast(R32),
                x_d[ds(c * 2048, 2048), :].bitcast(R32).rearrange(
                    "(j p) d -> p j d", p=128))
        nc.gpsimd.memset(mms[:], -3.0e38)
        nc.gpsimd.memset(nba[:], -(127.0 - 0.0435) * 0.6931472)
        nc.gpsimd.memset(msum[:], 1.0e-30)
        # r32 memset fails the ISA check; copy a DMA-loaded ones tile
        nc.vector.tensor_copy(rb(xV[:, :, D : D + 1]), onesb[:, :].rearrange("p (j o) -> p j o", o=1))

        # ---- g-proj + g8 (xqT arrives pre-transposed) ----
        for s in range(8):
            pm = psAr[s % 2][0:D, 0:256]
            nc.tensor.matmul(pm, rb(gmat[:]), rb(xqT[:, ts(s, 256)]),
                             start=True, stop=True)
            nc.vector.tensor_copy(rb(gB[0:D, ts(s, 256)]), pm)
        nc.scalar.copy(g8[:, 0, :], gB[0:32, :])
        nc.scalar.copy(g8[:, 1, :], gB[32:64, :])

        # ---- pass-A plumbing (512-key chunks on a 3-tile ring) ----
        a_ring = [0]

        LSE_CHUNKS = (1, 3, 5)   # handled by ACT as sum(exp(s/8))

        def emit_A(t, c, allow_lse=True):
            r = psAr[a_ring[0] % 3]
            a_ring[0] += 1
            nc.tensor.matmul(r[:], g8[:, :, ts(t, 128)],
                             x8[:, :, ds(c * 512, 512)],
                             start=True, stop=True, perf_mode=DR)
            if allow_lse and c in LSE_CHUNKS:
                # ACT: accum = sum exp(s/8); 8*log(sum) bounds the chunk max
                # within +8*ln(512). Keeps ~10% of the max scan off DVE.
                ls = scrp.tile([128, 512], F32, tag="lscr")
                nc.scalar.activation(ls[:], r[:],
                                     mybir.ActivationFunctionType.Exp,
                                     scale=0.125,
                                     accum_out=msum[:, t, LSE_CHUNKS.index(c)
                                                    : LSE_CHUNKS.index(c) + 1])
            else:
                nc.vector.reduce_max(mms[:, t, c : c + 1], r[:], axis=AX)

        SCHRA_A = 2.0 ** 23 / 0.6931472          # Schraudolph scale
        SCHRA_BA = (127.0 - 0.0435) * 0.6931472   # B/A bias (ln-space)

        def emit_mfin(t, bias=0.0):
            mt = smp.tile([128, 1], F32, tag="mt")
            nc.vector.reduce_max(mt[:], mms[:, t, :], axis=AX)
            # fold in the lse cells: m2 = 8*ln(max sums) via the bitcast
            # log2 approximation (error well inside the bound slack)
            sm = smp.tile([128, 1], F32, tag="sm")
            nc.vector.reduce_max(sm[:], msum[:, t, :], axis=AX)
            smf = smp.tile([128, 1], F32, tag="smf")
            nc.vector.tensor_copy(smf[:], sm[:].bitcast(mybir.dt.int32))
            nc.vector.tensor_scalar(smf[:], smf[:], 8 * 0.6931472 / 2 ** 23,
                                    -126.9 * 8 * 0.6931472,
                                    op0=mybir.AluOpType.mult,
                                    op1=mybir.AluOpType.add)
            nc.vector.tensor_tensor(mt[:], mt[:], smf[:],
                                    op=mybir.AluOpType.max)
            # pneg = mt^T @ (-I) = -m̂ row; runs as a psA-ring op so no
            # live psum region is disturbed (a start=True matmul must not
            # share a bank with an in-flight accumulation group).
            r = psAr[a_ring[0] % 3]
            a_ring[0] += 1
            pneg = r[0:1, 0:128]
            nc.tensor.matmul(pneg, mt[:], nident[:], start=True, stop=True)
            if bias == 0.0:
                nc.vector.tensor_copy(rb(gB[D : D + 1, ts(t, 128)]), pneg)
            else:
                nc.vector.tensor_scalar(rb(gB[D : D + 1, ts(t, 128)]), pneg,
                                        bias, None, op0=mybir.AluOpType.add)

        # ---- setup: kv chunks -> xT, x8, pass A for group 0 ----
        for c in range(8):
            for oo in range(2):
                o = 2 * c + oo
                tgt = psBs[o % 2]
                for j in range(8):
                    nc.tensor.transpose(tgt[0 : D + 1, ts(j, 128)],
                                        xV[:, o * 8 + j, :], ident[:])
                nc.scalar.copy(rb(xT[:, ts(o, 1024)]), tgt[0 : D + 1, :])
                nc.scalar.copy(x8[:, 0, ts(o, 1024)], tgt[0:32, :])
                # Pool cannot read PSUM: slot-1 shadow reads xT after its copy
                nc.gpsimd.tensor_copy(x8[:, 1, ts(o, 1024)],
                                      xT[32:64, ts(o, 1024)])
                # group-0 A work: oct o = keys [1024*o, 1024*o+1024):
                # chunks 2o, 2o+1 for tiles 0 and 1, ring-interleaved.
                for k in (2 * o, 2 * o + 1):
                    emit_A(0, k)
                    emit_A(1, k)
        emit_mfin(0)
        emit_mfin(1)

        # ---- phases ----
        for h in range(N_PHASE):
            qsl = ds(h * 256, 256)
            atiles = (2 * (h + 1), 2 * (h + 1) + 1) if h < N_PHASE - 1 else None
            pv_q = []   # batches of (blk, ex_ap); popped two units later
            # A-work queue: per tile 8 big + 16 sm ops; interleave
            # big/sm so the single-buffered tiles get WAR slack. Front-load
            # slightly so mfin chains clear before the next phase starts.
            a_q = []
            if atiles is not None:
                a_q = [(t, c) for t in atiles for c in range(32)]
            a_done = {}

            def emit_exp(u):
                ex = expp.tile([128, 1024], F32, tag="ex")
                nc.scalar.activation(rb(ex[:]), psBs[u % 2][:],
                                     mybir.ActivationFunctionType.Exp)
                pv_q.append([(4 * u + j, ex[:, ds(j * 256, 256)])
                             for j in range(4)])

            def emit_pv():
                for blk, ex_ap in pv_q.pop(0):
                    nc.tensor.matmul(psPV[0 : D + 1, 0:256], rb(xV[:, blk, :]),
                                     rb(ex_ap), start=(blk == 0),
                                     stop=(blk == N_KV_BLK - 1),
                                     skip_group_check=True)

            for u in range(N_UNIT):
                # Bs first: they feed this unit's exp with no other deps.
                for j in range(4):
                    blk = 4 * u + j
                    nc.tensor.matmul(psBs[u % 2][:, ds(j * 256, 256)],
                                     rb(xT[:, ts(blk, 128)]), rb(gB[:, qsl]),
                                     start=True, stop=True)
                # PVs of unit u-2: their exp finished during unit u-1.
                if len(pv_q) >= 2:
                    emit_pv()
                n_a = 0 if u == 0 else (3 if u < 6 else 2)
                for _ in range(min(n_a, len(a_q))):
                    t, k = a_q.pop(0)
                    emit_A(t, k)
                    a_done[t] = a_done.get(t, 0) + 1
                    if a_done[t] == 32:
                        emit_mfin(t)
                emit_exp(u)
            # tail: mfin chains are already emitted; flush PVs, save out1
            while pv_q:
                emit_pv()
            nc.scalar.copy(rb(OT[:, qsl]), psPV[0 : D + 1, 0:256])
            # [Wv|bv]-stage for this group, written back into OT rows 0:64;
            # then the denominator row becomes its reciprocal in place, so
            # the final transpose carries 1/denom in column 64. Exact fp32:
            # 8 matmuls of 256 cols, precision is worth the 4x row cost.
            pm = psAr[2][0:D, 256:512]
            nc.tensor.matmul(pm, wvb[:], OT[:, qsl],
                             start=True, stop=True)
            nc.scalar.copy(rb(OT[0:D, qsl]), pm)
            # reciprocal can't emit fp32r; bounce through scratch + rb copy
            rrow = scrp.tile([128, 256], F32, tag="scr")
            nc.vector.reciprocal(rrow[0:1, :], OT[D : D + 1, qsl])
            nc.vector.tensor_copy(rb(OT[D : D + 1, qsl]), rrow[0:1, :])

        # ---- final: transpose OT (col 64 = 1/denom), normalize, store ----
        for t in range(N_QTILE):
            pO = psBs[t % 2][:, 0 : D + 1]
            nc.tensor.transpose(pO, OT[:, ts(t, 128)],
                                ident[0 : D + 1, 0 : D + 1])
            nc.vector.tensor_scalar_mul(y_sb[:, t, :], pO[:, 0:D],
                                        pO[:, D : D + 1])
            if t % 4 == 3:
                nc.sync.dma_start(
                    y_d[ds((t - 3) * 128, 512), :].rearrange(
                        "(t p) d -> p t d", p=128),
                    y_sb[:, ds(t - 3, 4), :])

    nc.compile()
    return nc


def _prep_inputs(x, params, Wq, bq, Wk, bk, Wv, bv):
    f8s = np.float64
    x = np.ascontiguousarray(x, dtype=np.float32)
    params = np.asarray(params, f8s)
    rot = params[:, :D]
    ent = params[:, D : 2 * D]
    scale = 1.0 / np.sqrt(D)
    Wqp = (np.asarray(Wq, f8s) @ rot) * scale
    Wkp = np.asarray(Wk, f8s) @ ent
    bqs = np.asarray(bq, f8s) * scale
    G = Wkp.T @ Wqp
    w = Wkp.T @ bqs
    gmat = np.ascontiguousarray(
        np.vstack([G.T, w[None, :]]).astype(np.float32))
    wvb = np.ascontiguousarray(
        np.vstack([np.asarray(Wv, np.float32).T,
                   np.asarray(bv, np.float32)[None]]))
    ident = np.eye(128, dtype=np.float32)
    return x, gmat, wvb, ident


def kernel(x, params, Wq, bq, Wk, bk, Wv, bv, _trace=False):
    x, gmat, wvb, ident = _prep_inputs(x, params, Wq, bq, Wk, bk, Wv, bv)
    if "nc" not in _CACHED:
        _CACHED["nc"] = build_kernel()
    nc = _CACHED["nc"]
    in_maps = []
    for c in range(N_CORES):
        in_maps.append({
            "x": x,
            "xq": np.ascontiguousarray(np.vstack(
                [x[c * QR : (c + 1) * QR].T,
                 np.ones([1, QR], np.float32)])),
            "gmat": gmat, "wvb": wvb, "ident": ident, "nident": -ident,
            "onescol": np.ones([128, 128], np.float32),
        })
    res = run_bass_kernel_spmd(nc, in_maps, core_ids=list(range(N_CORES)),
                               trace=_trace)
    out = np.concatenate([res.results[c]["y"] for c in range(N_CORES)], axis=0)
    global _CACHED_RES
    _CACHED_RES = res
    return out


# revision 56
# speedup vs baseline: 1.1819x; 1.0139x over previous
"""Classical self-attention on 8 trn2 NeuronCores — v3.

N=16384 tokens, d=64, fp32. Sequence-parallel over Q: core c handles rows
[c*2048, (c+1)*2048). Per-core differentiation via the xq input slice.

Algebra (host-folded):
  s_jq = k_j . q_q = x_j^T (G x_q + w) + const_q, G/w precomputed on host;
  the per-q const is absorbed by the softmax shift, so it is never computed.
  sum_j p_j v_j = [Wv|bv] @ (sum_j p_j [x_j;1])  -> V never materialized;
  the ones column of xh doubles as the denominator row.

Structure per core:
  setup: DMA x (8 chunks) + xq; PE-transpose x -> xT[65,16384] (ones row);
    fp8 shadows x8[32,2,N] (DoubleRow d-split: d = slot*32 + partition);
    g-proj gB[65,2048] = [G^T;w^T] @ xqT, fp8 shadow g8; pass A for group 0.
  pass A (row max): fp8 DR matmuls psA[128,1024] per (tile, 1024-key chunk);
    DVE tensor_tensor_reduce (max,max) -> mms[:,t,c]; mfin: reduce -> mt,
    PE transpose -> gB row 64 = -m̂.
  8 phases (groups of 256 q = 2 tiles): per unit (4 kv blocks):
    PV(u-2) x4, B x4 (fp32r, psB[:,u%2,:]), exp(u-1) on ACT, TTR(u-1),
    A-mm(u) for group h+1 last (spacing hides the single-buffered psA WAR).
  tail: psPV[65,256] -> OT; after phase 7: OTw = [Wv|bv]^T @ OT (+denom row),
    PE transpose, DVE reciprocal+scale, DMA out.
"""

import sys

sys.path.insert(0, "/opt/trn_rl_repo")

from contextlib import ExitStack

import numpy as np

import concourse.bass as bass
import concourse.mybir as mybir
import concourse.tile as tile
from concourse import bacc
from concourse.bass import ds, ts
from concourse.bass_utils import run_bass_kernel_spmd

N_CORES = 8
N = 16384
D = 64
QR = N // N_CORES          # 2048 q rows per core
N_QTILE = QR // 128        # 16 q tiles per core
N_PHASE = 8                # groups of 2 q-tiles (256 q cols)
N_KV_BLK = N // 128        # 128 kv blocks
N_UNIT = 32                # units per phase (4 blocks each)
N_ACH = 16                 # 1024-key A-chunks per q tile
F32 = mybir.dt.float32
F8 = mybir.dt.float8e4
R32 = mybir.dt.float32r
DR = mybir.MatmulPerfMode.DoubleRow
AX = mybir.AxisListType.X
MAX = mybir.AluOpType.max

_CACHED = {}


def build_kernel():
    nc = bacc.Bacc("TRN2", target_bir_lowering=False, debug=False,
                   num_devices=N_CORES)

    x_d = nc.dram_tensor("x", [N, D], F32, kind="ExternalInput")
    xq_d = nc.dram_tensor("xq", [D + 1, QR], F32, kind="ExternalInput")
    gm_d = nc.dram_tensor("gmat", [D + 1, D], F32, kind="ExternalInput")
    wv_d = nc.dram_tensor("wvb", [D + 1, D], F32, kind="ExternalInput")
    id_d = nc.dram_tensor("ident", [128, 128], F32, kind="ExternalInput")
    nid_d = nc.dram_tensor("nident", [128, 128], F32, kind="ExternalInput")
    oc_d = nc.dram_tensor("onescol", [128, 128], F32, kind="ExternalInput")
    or_d = nc.dram_tensor("onesrow", [2, N // 2], F32, kind="ExternalInput")
    y_d = nc.dram_tensor("y", [QR, D], F32, kind="ExternalOutput")

    def rb(ap):
        return ap.bitcast(R32)

    with tile.TileContext(nc) as tc, ExitStack() as ctx:
        sb = ctx.enter_context(tc.tile_pool(name="sb", bufs=1))
        expp = ctx.enter_context(tc.tile_pool(name="expp", bufs=4))
        smp = ctx.enter_context(tc.tile_pool(name="smp", bufs=4))
        scrp = ctx.enter_context(tc.tile_pool(name="scr", bufs=2))
        psB_p = ctx.enter_context(tc.tile_pool(name="psB", bufs=1, space="PSUM"))
        psA_p = ctx.enter_context(tc.tile_pool(name="psA", bufs=1, space="PSUM"))
        psPV_p = ctx.enter_context(tc.tile_pool(name="psPV", bufs=1, space="PSUM"))
        psM_p = ctx.enter_context(tc.tile_pool(name="psM", bufs=1, space="PSUM"))

        # ---- persistent SBUF ----
        xT = sb.tile([D + 1, N], F32)          # x^T, row 64 = ones
        x8 = sb.tile([32, 2, N], F8)           # DR d-split shadow of xT
        xV = sb.tile([128, N_KV_BLK, D + 1], F32)  # natural x, col 64 = ones
        xqT = sb.tile([D + 1, QR], F32)        # xq^T, row 64 = ones
        gB = sb.tile([D + 1, QR], F32)         # g, row 64 = -m̂
        g8 = sb.tile([32, 2, QR], F8)
        gmat = sb.tile([D + 1, D], F32)
        wvb = sb.tile([D + 1, D], F32)
        ident = sb.tile([128, 128], F32)
        nident = sb.tile([128, 128], F32)
        onesb = sb.tile([128, 128], F32)
        nba = sb.tile([128, 1], F32)     # -B/A bias for phase-7 ACT exps
        mms = sb.tile([128, N_QTILE, 32], F32)
        msum = sb.tile([128, N_QTILE, 4], F32)   # ACT-lse partial sums
        # OT rows 0:64: out1 = sum p x, overwritten in-place by the
        # [Wv|bv]-stage at each phase tail; row 64 = denominator (sum p).
        OT = sb.tile([D + 1, QR], F32)
        y_sb = sb.tile([128, N_QTILE, D], F32)

        # ---- PSUM ----
        # The tile framework tracks PSUM hazards at tile granularity (and
        # marks PSUM-reading ACT/DVE ops as writers), so double-buffering
        # must use SEPARATE tiles, not halves of one tile.
        psBs = [psB_p.tile([128, 1024], F32, tag="psba", name="psba"),
                psB_p.tile([128, 1024], F32, tag="psbb", name="psbb")]
        # three-tile psA ring: reuse distance 3 keeps the single-tile
        # WAR chains from blocking the PE queue behind the DVE backlog.
        psAr = [psA_p.tile([128, 512], F32, tag=f"psa{i}", name=f"psa{i}")
                for i in range(3)]
        # psPV bank also hosts pneg (partition 127) - disjoint regions of
        # one tile; tile-granular hazards only serialize phase tails.
        psPV = psPV_p.tile([128, 512], F32)

        # ---- DMAs (DMA_ENGINES serializes; order by first use) ----
        nc.gpsimd.dma_start(ident[:], id_d[:])
        nc.gpsimd.dma_start(gmat[:].bitcast(R32), gm_d[:].bitcast(R32))
        nc.gpsimd.dma_start(wvb[:], wv_d[:])
        nc.gpsimd.dma_start(nident[:], nid_d[:])
        nc.gpsimd.dma_start(onesb[:], oc_d[:])
        nc.sync.dma_start(xqT[:].bitcast(R32), xq_d[:].bitcast(R32))
        nc.sync.dma_start(xT[D : D + 1, 0 : N // 2].bitcast(R32),
                          or_d[0:1, :].bitcast(R32))
        nc.sync.dma_start(xT[D : D + 1, N // 2 : N].bitcast(R32),
                          or_d[1:2, :].bitcast(R32))
        for c in range(8):
            nc.sync.dma_start(
                xV[:, ds(c * 16, 16), 0:D].bitcast(R32),
                x_d[ds(c * 2048, 2048), :].bitcast(R32).rearrange(
                    "(j p) d -> p j d", p=128))
        nc.gpsimd.memset(mms[:], -3.0e38)
        nc.gpsimd.memset(nba[:], -(127.0 - 0.0435) * 0.6931472)
        nc.gpsimd.memset(msum[:], 1.0e-30)
        # r32 memset fails the ISA check; copy a DMA-loaded ones tile
        nc.vector.tensor_copy(rb(xV[:, :, D : D + 1]), onesb[:, :].rearrange("p (j o) -> p j o", o=1))

        # ---- g-proj + g8 (xqT arrives pre-transposed) ----
        for s in range(8):
            pm = psAr[s % 2][0:D, 0:256]
            nc.tensor.matmul(pm, rb(gmat[:]), rb(xqT[:, ts(s, 256)]),
                             start=True, stop=True)
            nc.vector.tensor_copy(rb(gB[0:D, ts(s, 256)]), pm)
        nc.scalar.copy(g8[:, 0, :], gB[0:32, :])
        nc.scalar.copy(g8[:, 1, :], gB[32:64, :])

        # ---- pass-A plumbing (512-key chunks on a 3-tile ring) ----
        a_ring = [0]

        LSE_CHUNKS = (1, 3, 5)   # handled by ACT as sum(exp(s/8))

        def emit_A(t, c, allow_lse=True):
            r = psAr[a_ring[0] % 3]
            a_ring[0] += 1
            nc.tensor.matmul(r[:], g8[:, :, ts(t, 128)],
                             x8[:, :, ds(c * 512, 512)],
                             start=True, stop=True, perf_mode=DR)
            if allow_lse and c in LSE_CHUNKS:
                # ACT: accum = sum exp(s/8); 8*log(sum) bounds the chunk max
                # within +8*ln(512). Keeps ~10% of the max scan off DVE.
                ls = scrp.tile([128, 512], F32, tag="lscr")
                nc.scalar.activation(ls[:], r[:],
                                     mybir.ActivationFunctionType.Exp,
                                     scale=0.125,
                                     accum_out=msum[:, t, LSE_CHUNKS.index(c)
                                                    : LSE_CHUNKS.index(c) + 1])
            else:
                nc.vector.reduce_max(mms[:, t, c : c + 1], r[:], axis=AX)

        SCHRA_A = 2.0 ** 23 / 0.6931472          # Schraudolph scale
        SCHRA_BA = (127.0 - 0.0435) * 0.6931472   # B/A bias (ln-space)

        def emit_mfin(t, bias=0.0):
            mt = smp.tile([128, 1], F32, tag="mt")
            nc.vector.reduce_max(mt[:], mms[:, t, :], axis=AX)
            # fold in the lse cells: m2 = 8*ln(max sums) via the bitcast
            # log2 approximation (error well inside the bound slack)
            sm = smp.tile([128, 1], F32, tag="sm")
            nc.vector.reduce_max(sm[:], msum[:, t, :], axis=AX)
            smf = smp.tile([128, 1], F32, tag="smf")
            nc.vector.tensor_copy(smf[:], sm[:].bitcast(mybir.dt.int32))
            nc.vector.tensor_scalar(smf[:], smf[:], 8 * 0.6931472 / 2 ** 23,
                                    -126.9 * 8 * 0.6931472,
                                    op0=mybir.AluOpType.mult,
                                    op1=mybir.AluOpType.add)
            nc.vector.tensor_tensor(mt[:], mt[:], smf[:],
                                    op=mybir.AluOpType.max)
            # pneg = mt^T @ (-I) = -m̂ row; runs as a psA-ring op so no
            # live psum region is disturbed (a start=True matmul must not
            # share a bank with an in-flight accumulation group).
            r = psAr[a_ring[0] % 3]
            a_ring[0] += 1
            pneg = r[0:1, 0:128]
            nc.tensor.matmul(pneg, mt[:], nident[:], start=True, stop=True)
            if bias == 0.0:
                nc.vector.tensor_copy(rb(gB[D : D + 1, ts(t, 128)]), pneg)
            else:
                nc.vector.tensor_scalar(rb(gB[D : D + 1, ts(t, 128)]), pneg,
                                        bias, None, op0=mybir.AluOpType.add)

        # ---- setup: kv chunks -> xT, x8, pass A for group 0 ----
        for c in range(8):
            for oo in range(2):
                o = 2 * c + oo
                tgt = psBs[o % 2]
                for j in range(8):
                    nc.tensor.transpose(tgt[0:D, ts(j, 128)],
                                        xV[:, o * 8 + j, 0:D], ident[:])
                nc.scalar.copy(rb(xT[0:D, ts(o, 1024)]), tgt[0:D, :])
                nc.scalar.copy(x8[:, 0, ts(o, 1024)], tgt[0:32, :])
                # Pool cannot read PSUM: slot-1 shadow reads xT after its copy
                nc.gpsimd.tensor_copy(x8[:, 1, ts(o, 1024)],
                                      xT[32:64, ts(o, 1024)])
                # group-0 A work: oct o = keys [1024*o, 1024*o+1024):
                # chunks 2o, 2o+1 for tiles 0 and 1, ring-interleaved.
                for k in (2 * o, 2 * o + 1):
                    emit_A(0, k)
                    emit_A(1, k)
        emit_mfin(0)
        emit_mfin(1)

        # ---- phases ----
        for h in range(N_PHASE):
            qsl = ds(h * 256, 256)
            atiles = (2 * (h + 1), 2 * (h + 1) + 1) if h < N_PHASE - 1 else None
            pv_q = []   # batches of (blk, ex_ap); popped two units later
            # A-work queue: per tile 8 big + 16 sm ops; interleave
            # big/sm so the single-buffered tiles get WAR slack. Front-load
            # slightly so mfin chains clear before the next phase starts.
            a_q = []
            if atiles is not None:
                a_q = [(t, c) for t in atiles for c in range(32)]
            a_done = {}

            def emit_exp(u):
                ex = expp.tile([128, 1024], F32, tag="ex")
                nc.scalar.activation(rb(ex[:]), psBs[u % 2][:],
                                     mybir.ActivationFunctionType.Exp)
                pv_q.append([(4 * u + j, ex[:, ds(j * 256, 256)])
                             for j in range(4)])

            def emit_pv():
                for blk, ex_ap in pv_q.pop(0):
                    nc.tensor.matmul(psPV[0 : D + 1, 0:256], rb(xV[:, blk, :]),
                                     rb(ex_ap), start=(blk == 0),
                                     stop=(blk == N_KV_BLK - 1),
                                     skip_group_check=True)

            for u in range(N_UNIT):
                # Bs first: they feed this unit's exp with no other deps.
                for j in range(4):
                    blk = 4 * u + j
                    nc.tensor.matmul(psBs[u % 2][:, ds(j * 256, 256)],
                                     rb(xT[:, ts(blk, 128)]), rb(gB[:, qsl]),
                                     start=True, stop=True)
                # PVs of unit u-2: their exp finished during unit u-1.
                if len(pv_q) >= 2:
                    emit_pv()
                n_a = 0 if u == 0 else (3 if u < 6 else 2)
                for _ in range(min(n_a, len(a_q))):
                    t, k = a_q.pop(0)
                    emit_A(t, k)
                    a_done[t] = a_done.get(t, 0) + 1
                    if a_done[t] == 32:
                        emit_mfin(t)
                emit_exp(u)
            # tail: mfin chains are already emitted; flush PVs, save out1
            while pv_q:
                emit_pv()
            nc.scalar.copy(rb(OT[:, qsl]), psPV[0 : D + 1, 0:256])
            # [Wv|bv]-stage for this group, written back into OT rows 0:64;
            # then the denominator row becomes its reciprocal in place, so
            # the final transpose carries 1/denom in column 64. Exact fp32:
            # 8 matmuls of 256 cols, precision is worth the 4x row cost.
            pm = psAr[2][0:D, 256:512]
            nc.tensor.matmul(pm, wvb[:], OT[:, qsl],
                             start=True, stop=True)
            nc.scalar.copy(rb(OT[0:D, qsl]), pm)
            # reciprocal can't emit fp32r; bounce through scratch + rb copy
            rrow = scrp.tile([128, 256], F32, tag="scr")
            nc.vector.reciprocal(rrow[0:1, :], OT[D : D + 1, qsl])
            nc.vector.tensor_copy(rb(OT[D : D + 1, qsl]), rrow[0:1, :])

        # ---- final: transpose OT (col 64 = 1/denom), normalize, store ----
        for t in range(N_QTILE):
            pO = psBs[t % 2][:, 0 : D + 1]
            nc.tensor.transpose(pO, OT[:, ts(t, 128)],
                                ident[0 : D + 1, 0 : D + 1])
            nc.vector.tensor_scalar_mul(y_sb[:, t, :], pO[:, 0:D],
                                        pO[:, D : D + 1])
            if t % 4 == 3:
                nc.sync.dma_start(
                    y_d[ds((t - 3) * 128, 512), :].rearrange(
                        "(t p) d -> p t d", p=128),
                    y_sb[:, ds(t - 3, 4), :])

    nc.compile()
    return nc


def _prep_inputs(x, params, Wq, bq, Wk, bk, Wv, bv):
    f8s = np.float64
    x = np.ascontiguousarray(x, dtype=np.float32)
    params = np.asarray(params, f8s)
    rot = params[:, :D]
    ent = params[:, D : 2 * D]
    scale = 1.0 / np.sqrt(D)
    Wqp = (np.asarray(Wq, f8s) @ rot) * scale
    Wkp = np.asarray(Wk, f8s) @ ent
    bqs = np.asarray(bq, f8s) * scale
    G = Wkp.T @ Wqp
    w = Wkp.T @ bqs
    gmat = np.ascontiguousarray(
        np.vstack([G.T, w[None, :]]).astype(np.float32))
    wvb = np.ascontiguousarray(
        np.vstack([np.asarray(Wv, np.float32).T,
                   np.asarray(bv, np.float32)[None]]))
    ident = np.eye(128, dtype=np.float32)
    return x, gmat, wvb, ident


def kernel(x, params, Wq, bq, Wk, bk, Wv, bv, _trace=False):
    x, gmat, wvb, ident = _prep_inputs(x, params, Wq, bq, Wk, bk, Wv, bv)
    if "nc" not in _CACHED:
        _CACHED["nc"] = build_kernel()
    nc = _CACHED["nc"]
    in_maps = []
    for c in range(N_CORES):
        in_maps.append({
            "x": x,
            "xq": np.ascontiguousarray(np.vstack(
                [x[c * QR : (c + 1) * QR].T,
                 np.ones([1, QR], np.float32)])),
            "gmat": gmat, "wvb": wvb, "ident": ident, "nident": -ident,
            "onescol": np.ones([128, 128], np.float32),
            "onesrow": np.ones([2, N // 2], np.float32),
        })
    res = run_bass_kernel_spmd(nc, in_maps, core_ids=list(range(N_CORES)),
                               trace=_trace)
    out = np.concatenate([res.results[c]["y"] for c in range(N_CORES)], axis=0)
    global _CACHED_RES
    _CACHED_RES = res
    return out
